# revision 2
# baseline (speedup 1.0000x reference)
"""GQA attention kernel for Trainium2, tensor-parallel across 8 NeuronCores.

Problem: B=2, T=2048, D=2048, H=32 q-heads, G=8 kv-heads (GQA, rep=4), hd=64,
causal softmax attention + output projection, fp32 I/O.

Sharding (one KV group per core):
  core g: Wq[:, g*256:(g+1)*256], Wk/Wv[:, g*64:(g+1)*64], Wo[g*256:(g+1)*256, :]
  Each core computes its 4 heads' attention + partial output projection;
  host sums the 8 partial outputs (row-parallel Wo => partial-sum unshard).
  Partial outputs are stored bf16 (halves store bandwidth); host sums in f32.

On-device dataflow per core (PE cost model: out_free_size cycles per matmul):
  QT = wq.T @ xT        [256, T] fp32r (1/8 scale folded into psum->sbuf copy)
  [KT; VT] = wkv.T @ xT [128, T]; KT duplicated to partitions 64..127,
  VT -> bf16 -> per-kt DMA-transpose into V1 [kpos, 16, hd|1] with ones col.
  Scores per (pair, kt): two matmuls (head halves) into a 2-bank psum
  [128, 2, 512]; ONE exp -> P[pair] sbuf bf16 [128, 2, nkt, 512].
  Diagonal kt: matmul/exp trimmed to columns >= min(off, 256); triangle
  masked by multiplying with a precomputed upper-tri bf16 mask (DVE).
  PV: per (half, j): out[q=128, 65] psum accumulated over kt with P as
  lhsT (65 cycles/matmul instead of 512) against V1[kpos, 65].
  Normalize: reciprocal of col 64 ([128,1]) * out cols 0..63 -> o_n bf16.
  O transpose via DMA-transpose: o_n [128q, 2*64] -> ot [128 dq, 128 q].
  Wo partial: ot.T @ wo (bf16) -> psum [128, 512] x4 nb -> stg bf16
  [128, 2048] -> one DMA per 128-token row block.
"""

import os
import sys

import numpy as np

for _p in ("/opt/trn_rl_repo", "/root/.axon_site/_ro/trn_rl_repo"):
    if os.path.isdir(_p) and _p not in sys.path:
        sys.path.insert(0, _p)

import ml_dtypes  # noqa: E402

import concourse.bass as bass  # noqa: E402
import concourse.mybir as mybir  # noqa: E402
import concourse.tile as tile  # noqa: E402
from concourse import bacc  # noqa: E402
from concourse.bass_utils import run_bass_kernel_spmd  # noqa: E402
from concourse.masks import make_identity  # noqa: E402
from contextlib import ExitStack  # noqa: E402

B, T, D = 2, 2048, 2048
G, REP, HD = 8, 4, 64
DQ = REP * HD  # 256 q-dims per core
NCORES = 8
P = 128
TB = 512  # q/t block size
KO = D // P  # 16 contraction subtiles for projections
KQ = 4  # ko tiles per x DMA load
NT = T // TB  # 4 t-blocks
NKT = T // P  # 16 kpos tiles
F32 = mybir.dt.float32
F32R = mybir.dt.float32r
BF16 = mybir.dt.bfloat16
AF = mybir.ActivationFunctionType
SCALE = 1.0 / 8.0  # 1/sqrt(HD)


def build_kernel(ctx, tc):
    nc = tc.nc
    xT = nc.dram_tensor("xT", [B, D, T], BF16, kind="ExternalInput").ap()
    wq = nc.dram_tensor("wq", [D, DQ], BF16, kind="ExternalInput").ap()
    wkv = nc.dram_tensor("wkv", [D, 2 * HD], BF16, kind="ExternalInput").ap()
    wo = nc.dram_tensor("wo", [DQ, D], BF16, kind="ExternalInput").ap()
    out = nc.dram_tensor("out", [B, T, D], BF16, kind="ExternalOutput").ap()

    wpool = ctx.enter_context(tc.tile_pool(name="w", bufs=1))
    qt_pool = ctx.enter_context(tc.tile_pool(name="qt", bufs=2))
    kkt_pool = ctx.enter_context(tc.tile_pool(name="kkt", bufs=2))
    vt_pool = ctx.enter_context(tc.tile_pool(name="vt", bufs=2))
    v_pool = ctx.enter_context(tc.tile_pool(name="v", bufs=2))
    xt_pool = ctx.enter_context(tc.tile_pool(name="xt", bufs=3))
    p_pool = ctx.enter_context(tc.tile_pool(name="p", bufs=2))
    on_pool = ctx.enter_context(tc.tile_pool(name="on", bufs=3))
    rc_pool = ctx.enter_context(tc.tile_pool(name="rc", bufs=3))
    ot_pool = ctx.enter_context(tc.tile_pool(name="ot", bufs=2))
    stg_pool = ctx.enter_context(tc.tile_pool(name="stg", bufs=2))
    pp = ctx.enter_context(tc.tile_pool(name="pp", bufs=2, space="PSUM"))

    # persistent weights (SP/HWDGE queue; Pool is reserved for xt loads).
    # wq/wkv split into ko-chunks so the first matmuls wait only on chunk 0.
    wq_sb = wpool.tile([P, KO, DQ], BF16, tag="wq")
    wkv_sb = wpool.tile([P, KO, 2 * HD], BF16, tag="wkv")
    wq_r = wq.rearrange("(ko p) m -> p ko m", p=P)
    wkv_r = wkv.rearrange("(ko p) m -> p ko m", p=P)
    for c in range(0, KO, KQ):
        nc.sync.dma_start(wq_sb[:, c : c + KQ, :], wq_r[:, c : c + KQ, :])
        nc.sync.dma_start(wkv_sb[:, c : c + KQ, :], wkv_r[:, c : c + KQ, :])
    wo_sb = wpool.tile([P, DQ // P, D], BF16, tag="wo")
    nc.sync.dma_start(wo_sb[:], wo.rearrange("(ko p) m -> p ko m", p=P))
    # upper-triangular causal mask (keep f >= p), two identical copies so one
    # tensor_tensor covers both head halves of a pair at once
    ident = wpool.tile([HD, HD], BF16, tag="ident")
    make_identity(nc, ident[:])
    tri = wpool.tile([P, 2, P], BF16, tag="tri")
    nc.gpsimd.memset(tri[:], 1.0)
    for h in range(2):
        nc.gpsimd.affine_select(
            out=tri[:, h, :],
            in_=tri[:, h, :],
            compare_op=mybir.AluOpType.is_ge,
            fill=0.0,
            base=0,
            channel_multiplier=-1,
            pattern=[[1, P]],
        )

    for b in range(B):
        qt_sb = qt_pool.tile([P, 2, T], BF16, tag="qt")  # QT, scaled by 1/8
        kkt_sb = kkt_pool.tile([P, T], BF16, tag="kkt")  # KT duplicated twice
        vt_sb = vt_pool.tile([HD, T], BF16, tag="vt")  # VT bf16 on 64 parts
        v1_sb = v_pool.tile([P, NKT, HD + 1], BF16, tag="v1")
        nc.gpsimd.memset(v1_sb[:, :, HD : HD + 1], 1.0)

        def proj(tb):
            # ---------------- projections for t-block tb ----------------
            ts = slice(tb * TB, (tb + 1) * TB)
            q_ps = pp.tile([P, 2, TB], F32, tag="S")
            kv_ps = pp.tile([P, TB], F32, tag="O")
            for ko in range(KO):
                if ko % KQ == 0:
                    xt = xt_pool.tile([P, KQ, TB], BF16, tag="xt")
                    nc.gpsimd.dma_start(
                        xt[:],
                        xT[b, ko * P : (ko + KQ) * P, ts].rearrange(
                            "(q p) t -> p q t", p=P
                        ),
                    )
                st, sp_ = (ko == 0), (ko == KO - 1)
                nc.tensor.matmul(
                    q_ps[:, 0, :], wq_sb[:, ko, 0:P], xt[:, ko % KQ, :],
                    start=st, stop=sp_,
                )
                nc.tensor.matmul(
                    q_ps[:, 1, :], wq_sb[:, ko, P:DQ], xt[:, ko % KQ, :],
                    start=st, stop=sp_,
                )
                nc.tensor.matmul(
                    kv_ps[:], wkv_sb[:, ko, :], xt[:, ko % KQ, :],
                    start=st, stop=sp_,
                )
            # qt copy split per pair so scores can start after half the copy
            for pair in range(2):
                nc.scalar.activation(
                    qt_sb[:, pair, ts], q_ps[:, pair, :], AF.Copy, scale=SCALE
                )
            nc.vector.tensor_copy(kkt_sb[0:HD, ts], kv_ps[0:HD, :])
            nc.vector.tensor_copy(vt_sb[:, ts], kv_ps[HD:P, :])
            # duplicate KT to partitions 64..127 (SBUF->SBUF DMA moves partitions)
            nc.sync.dma_start(kkt_sb[HD:P, ts], kkt_sb[0:HD, ts])
            # V transpose via PE identity matmul: [64, 128] -> [128, 64]
            # (the [64,128] xbar DMA-transpose corrupts data on HW; the
            # [128,128] o_n xbar transpose below is fine)
            for kt in range(4 * tb, 4 * tb + 4):
                tr_ps = pp.tile([P, HD], BF16, tag="W")
                nc.tensor.transpose(
                    tr_ps[:], vt_sb[:, kt * P : (kt + 1) * P], ident[:]
                )
                nc.vector.tensor_copy(v1_sb[:, kt, 0:HD], tr_ps[:])

        def attn(qb):
            # ------------- attention + output proj for q-block qb -----------
            nkt = 4 * (qb + 1)  # causal: kpos tiles 0..nkt-1
            ot_sb = ot_pool.tile([P, 2, TB], BF16, tag="ot")
            for pair in range(2):
                # --- phase A: scores + exp for both head halves, all kt ---
                p_sb = p_pool.tile([P, 2, nkt, TB], BF16, tag="P")
                for kt in range(nkt):
                    ks = slice(kt * P, (kt + 1) * P)
                    dk = kt - qb * 4
                    off = max(dk, 0) * P  # first potentially-valid column
                    offc = off  # computed column start (bf16: any free size ok)
                    s_ps = pp.tile([P, 2, TB], F32, tag="S")
                    qs = slice(qb * TB + offc, (qb + 1) * TB)
                    nc.tensor.matmul(
                        s_ps[:, 0, offc:],
                        kkt_sb[0:HD, ks],
                        qt_sb[0:HD, pair, qs],
                        start=True,
                        stop=True,
                        tile_position=(0, 0),
                    )
                    nc.tensor.matmul(
                        s_ps[:, 1, offc:],
                        kkt_sb[HD:P, ks],
                        qt_sb[HD:P, pair, qs],
                        start=True,
                        stop=True,
                        tile_position=(64, 0),
                    )
                    nc.scalar.activation(
                        p_sb[:, :, kt, offc:], s_ps[:, :, offc:], AF.Exp
                    )
                    if dk >= 0:  # diagonal block: causal triangle mask
                        nc.vector.tensor_mul(
                            p_sb[:, :, kt, off : off + P],
                            p_sb[:, :, kt, off : off + P],
                            tri[:],
                        )
                # --- phase B: PV accumulation, normalize, transpose ---
                for j in range(NT):
                    ktn = qb * 4 + j + 1  # kpos tiles 0..ktn-1
                    o_n = on_pool.tile([P, 2, HD], BF16, tag="on")
                    for half in range(2):
                        o_ps = pp.tile([P, HD + 1], F32, tag="O")
                        for kt in range(ktn):
                            nc.tensor.matmul(
                                o_ps[:],
                                p_sb[:, half, kt, j * P : (j + 1) * P],
                                v1_sb[:, kt, :],
                                start=(kt == 0),
                                stop=(kt == ktn - 1),
                            )
                        rec = rc_pool.tile([P, 1], F32, tag="rec")
                        nc.vector.reciprocal(rec[:], o_ps[:, HD : HD + 1])
                        nc.vector.tensor_scalar_mul(
                            o_n[:, half, :], o_ps[:, 0:HD], rec[:]
                        )
                    nc.sync.dma_start_transpose(
                        ot_sb[:, pair, j * P : (j + 1) * P], o_n[:]
                    )
            # --- Wo partial for this q-block's 512 tokens ---
            for j in range(NT):
                rows = slice(qb * TB + j * P, qb * TB + (j + 1) * P)
                stg = stg_pool.tile([P, D], BF16, tag="stg")
                for nb in range(4):
                    wo_ps = pp.tile([P, TB], F32, tag="W")
                    for ko in range(2):
                        nc.tensor.matmul(
                            wo_ps[:],
                            ot_sb[:, ko, j * P : (j + 1) * P],
                            wo_sb[:, ko, nb * TB : (nb + 1) * TB],
                            start=(ko == 0),
                            stop=(ko == 1),
                        )
                    nc.vector.tensor_copy(stg[:, nb * TB : (nb + 1) * TB], wo_ps[:])
                nc.sync.dma_start(out[b, rows, :], stg[:])

        for tb in range(NT):
            proj(tb)
        for qb in range(NT):
            attn(qb)


_NC_CACHE = {}


def get_nc():
    if "nc" not in _NC_CACHE:
        nc = bacc.Bacc("TRN2", target_bir_lowering=False, debug=False)
        with tile.TileContext(nc) as tc, ExitStack() as ctx:
            build_kernel(ctx, tc)
        nc.compile()
        _NC_CACHE["nc"] = nc
    return _NC_CACHE["nc"]


def make_in_maps(x, Wq, Wk, Wv, Wo):
    xT = np.ascontiguousarray(
        np.transpose(np.asarray(x, np.float32), (0, 2, 1))
    ).astype(ml_dtypes.bfloat16)
    Wq, Wk, Wv, Wo = (np.asarray(w, np.float32) for w in (Wq, Wk, Wv, Wo))
    in_maps = []
    for g in range(NCORES):
        in_maps.append(
            {
                "xT": xT,
                "wq": np.ascontiguousarray(
                    Wq[:, g * DQ : (g + 1) * DQ]
                ).astype(ml_dtypes.bfloat16),
                "wkv": np.ascontiguousarray(
                    np.concatenate(
                        [Wk[:, g * HD : (g + 1) * HD], Wv[:, g * HD : (g + 1) * HD]],
                        axis=1,
                    )
                ).astype(ml_dtypes.bfloat16),
                "wo": np.ascontiguousarray(
                    Wo[g * DQ : (g + 1) * DQ, :]
                ).astype(ml_dtypes.bfloat16),
            }
        )
    return in_maps


def run(x, Wq, Wk, Wv, Wo, trace=False):
    nc = get_nc()
    in_maps = make_in_maps(x, Wq, Wk, Wv, Wo)
    res = run_bass_kernel_spmd(nc, in_maps, list(range(NCORES)), trace=trace)
    acc = np.zeros((B, T, D), np.float32)
    for r in res.results:
        acc += np.asarray(r["out"], dtype=np.float32)
    return acc, res


def kernel(x, Wq, Wk, Wv, Wo):
    return run(x, Wq, Wk, Wv, Wo)[0]


# revision 3
# speedup vs baseline: 1.0676x; 1.0676x over previous
"""GQA attention kernel for Trainium2, tensor-parallel across 8 NeuronCores.

Problem: B=2, T=2048, D=2048, H=32 q-heads, G=8 kv-heads (GQA, rep=4), hd=64,
causal softmax attention + output projection, fp32 I/O.

Sharding (one KV group per core):
  core g: Wq[:, g*256:(g+1)*256], Wk/Wv[:, g*64:(g+1)*64], Wo[g*256:(g+1)*256, :]
  Each core computes its 4 heads' attention + partial output projection;
  host sums the 8 partial outputs (row-parallel Wo => partial-sum unshard).
  Partial outputs are stored bf16 (halves store bandwidth); host sums in f32.

On-device dataflow per core (PE cost model: out_free_size cycles per matmul):
  QT = wq.T @ xT        [256, T] fp32r (1/8 scale folded into psum->sbuf copy)
  [KT; VT] = wkv.T @ xT [128, T]; KT duplicated to partitions 64..127,
  VT -> bf16 -> per-kt DMA-transpose into V1 [kpos, 16, hd|1] with ones col.
  Scores per (pair, kt): two matmuls (head halves) into a 2-bank psum
  [128, 2, 512]; ONE exp -> P[pair] sbuf bf16 [128, 2, nkt, 512].
  Diagonal kt: matmul/exp trimmed to columns >= min(off, 256); triangle
  masked by multiplying with a precomputed upper-tri bf16 mask (DVE).
  PV: per (half, j): out[q=128, 65] psum accumulated over kt with P as
  lhsT (65 cycles/matmul instead of 512) against V1[kpos, 65].
  Normalize: reciprocal of col 64 ([128,1]) * out cols 0..63 -> o_n bf16.
  O transpose via DMA-transpose: o_n [128q, 2*64] -> ot [128 dq, 128 q].
  Wo partial: ot.T @ wo (bf16) -> psum [128, 512] x4 nb -> stg bf16
  [128, 2048] -> one DMA per 128-token row block.
"""

import os
import sys

import numpy as np

for _p in ("/opt/trn_rl_repo", "/root/.axon_site/_ro/trn_rl_repo"):
    if os.path.isdir(_p) and _p not in sys.path:
        sys.path.insert(0, _p)

import ml_dtypes  # noqa: E402

import concourse.bass as bass  # noqa: E402
import concourse.mybir as mybir  # noqa: E402
import concourse.tile as tile  # noqa: E402
from concourse import bacc  # noqa: E402
from concourse.bass_utils import run_bass_kernel_spmd  # noqa: E402
from concourse.masks import make_identity  # noqa: E402
from contextlib import ExitStack  # noqa: E402

B, T, D = 2, 2048, 2048
G, REP, HD = 8, 4, 64
DQ = REP * HD  # 256 q-dims per core
NCORES = 8
P = 128
TB = 512  # q/t block size
KO = D // P  # 16 contraction subtiles for projections
KQ = 4  # ko tiles per x DMA load
NT = T // TB  # 4 t-blocks
NKT = T // P  # 16 kpos tiles
F32 = mybir.dt.float32
F32R = mybir.dt.float32r
BF16 = mybir.dt.bfloat16
AF = mybir.ActivationFunctionType
SCALE = 1.0 / 8.0  # 1/sqrt(HD)
PJ_BUFS = 1
S_BUFS = 2
O_BUFS = 2
W_BUFS = 1


def build_kernel(ctx, tc):
    nc = tc.nc
    xT = nc.dram_tensor("xT", [B, D, T], BF16, kind="ExternalInput").ap()
    wq = nc.dram_tensor("wq", [D, DQ], BF16, kind="ExternalInput").ap()
    wkv = nc.dram_tensor("wkv", [D, 2 * HD], BF16, kind="ExternalInput").ap()
    wo = nc.dram_tensor("wo", [DQ, D], BF16, kind="ExternalInput").ap()
    out = nc.dram_tensor("out", [B, T, D], BF16, kind="ExternalOutput").ap()

    wpool = ctx.enter_context(tc.tile_pool(name="w", bufs=1))
    qt_pool = ctx.enter_context(tc.tile_pool(name="qt", bufs=2))
    kkt_pool = ctx.enter_context(tc.tile_pool(name="kkt", bufs=2))
    vt_pool = ctx.enter_context(tc.tile_pool(name="vt", bufs=2))
    v_pool = ctx.enter_context(tc.tile_pool(name="v", bufs=2))
    xt_pool = ctx.enter_context(tc.tile_pool(name="xt", bufs=6))
    p_pool = ctx.enter_context(tc.tile_pool(name="p", bufs=2))
    on_pool = ctx.enter_context(tc.tile_pool(name="on", bufs=3))
    rc_pool = ctx.enter_context(tc.tile_pool(name="rc", bufs=3))
    ot_pool = ctx.enter_context(tc.tile_pool(name="ot", bufs=2))
    stg_pool = ctx.enter_context(tc.tile_pool(name="stg", bufs=2))
    pp = ctx.enter_context(tc.tile_pool(name="pp", bufs=2, space="PSUM"))

    # persistent weights (SP/HWDGE queue; Pool is reserved for xt loads).
    # wq/wkv split into ko-chunks so the first matmuls wait only on chunk 0.
    wq_sb = wpool.tile([P, KO, DQ], BF16, tag="wq")
    wkv_sb = wpool.tile([P, KO, 2 * HD], BF16, tag="wkv")
    wq_r = wq.rearrange("(ko p) m -> p ko m", p=P)
    wkv_r = wkv.rearrange("(ko p) m -> p ko m", p=P)
    for c in range(0, KO, KQ):
        nc.sync.dma_start(wq_sb[:, c : c + KQ, :], wq_r[:, c : c + KQ, :])
        nc.sync.dma_start(wkv_sb[:, c : c + KQ, :], wkv_r[:, c : c + KQ, :])
    wo_sb = wpool.tile([P, DQ // P, D], BF16, tag="wo")
    nc.sync.dma_start(wo_sb[:], wo.rearrange("(ko p) m -> p ko m", p=P))
    # upper-triangular causal mask (keep f >= p), two identical copies so one
    # tensor_tensor covers both head halves of a pair at once
    ident = wpool.tile([HD, HD], BF16, tag="ident")
    make_identity(nc, ident[:])
    tri = wpool.tile([P, 2, P], BF16, tag="tri")
    nc.gpsimd.memset(tri[:], 1.0)
    for h in range(2):
        nc.gpsimd.affine_select(
            out=tri[:, h, :],
            in_=tri[:, h, :],
            compare_op=mybir.AluOpType.is_ge,
            fill=0.0,
            base=0,
            channel_multiplier=-1,
            pattern=[[1, P]],
        )

    for b in range(B):
        qt_sb = qt_pool.tile([P, 2, T], BF16, tag="qt")  # QT, scaled by 1/8
        kkt_sb = kkt_pool.tile([P, T], BF16, tag="kkt")  # KT duplicated twice
        vt_sb = vt_pool.tile([HD, T], BF16, tag="vt")  # VT bf16 on 64 parts
        v1_sb = v_pool.tile([P, NKT, HD + 1], BF16, tag="v1")
        nc.gpsimd.memset(v1_sb[:, :, HD : HD + 1], 1.0)

        def proj(tb):
            # ---------------- projections for t-block tb ----------------
            ts = slice(tb * TB, (tb + 1) * TB)
            xts = []
            for kq in range(KO // KQ):
                xt = xt_pool.tile([P, KQ, TB], BF16, tag="xt")
                nc.gpsimd.dma_start(
                    xt[:],
                    xT[b, kq * KQ * P : (kq + 1) * KQ * P, ts].rearrange(
                        "(q p) t -> p q t", p=P
                    ),
                )
                xts.append(xt)
            # three sequential 1-bank accumulation chains (pair0, pair1, kv):
            # projection psum is a dedicated tag, so later-batch projections
            # can allocate independently of in-flight score tiles.
            for ci, (wsb, lo) in enumerate(
                ((wq_sb, 0), (wq_sb, P), (wkv_sb, 0))
            ):
                c_ps = pp.tile([P, TB], F32, tag="PJ", bufs=PJ_BUFS)
                for ko in range(KO):
                    nc.tensor.matmul(
                        c_ps[:],
                        wsb[:, ko, lo : lo + P],
                        xts[ko // KQ][:, ko % KQ, :],
                        start=(ko == 0),
                        stop=(ko == KO - 1),
                    )
                if ci < 2:
                    # on DVE, not ACT: keeps the exp queue free of copies
                    nc.vector.tensor_scalar_mul(qt_sb[:, ci, ts], c_ps[:], SCALE)
                else:
                    nc.vector.tensor_copy(kkt_sb[0:HD, ts], c_ps[0:HD, :])
                    nc.vector.tensor_copy(vt_sb[:, ts], c_ps[HD:P, :])
            # duplicate KT to partitions 64..127 (SBUF->SBUF DMA moves partitions)
            nc.sync.dma_start(kkt_sb[HD:P, ts], kkt_sb[0:HD, ts])
            # V transpose via PE identity matmul: [64, 128] -> [128, 64]
            # (the [64,128] xbar DMA-transpose corrupts data on HW; the
            # [128,128] o_n xbar transpose below is fine)
            for kt in range(4 * tb, 4 * tb + 4):
                tr_ps = pp.tile([P, HD], BF16, tag="W", bufs=W_BUFS)
                nc.tensor.transpose(
                    tr_ps[:], vt_sb[:, kt * P : (kt + 1) * P], ident[:]
                )
                nc.vector.tensor_copy(v1_sb[:, kt, 0:HD], tr_ps[:])

        def attn(qb):
            # ------------- attention + output proj for q-block qb -----------
            nkt = 4 * (qb + 1)  # causal: kpos tiles 0..nkt-1
            ot_sb = ot_pool.tile([P, 2, TB], BF16, tag="ot")
            for pair in range(2):
                # --- phase A: scores + exp for both head halves, all kt ---
                p_sb = p_pool.tile([P, 2, nkt, TB], BF16, tag="P")
                for kt in range(nkt):
                    ks = slice(kt * P, (kt + 1) * P)
                    dk = kt - qb * 4
                    off = max(dk, 0) * P  # first potentially-valid column
                    offc = off  # computed column start (bf16: any free size ok)
                    s_ps = pp.tile([P, 2, TB], F32, tag="S", bufs=S_BUFS)
                    qs = slice(qb * TB + offc, (qb + 1) * TB)
                    nc.tensor.matmul(
                        s_ps[:, 0, offc:],
                        kkt_sb[0:HD, ks],
                        qt_sb[0:HD, pair, qs],
                        start=True,
                        stop=True,
                        tile_position=(0, 0),
                    )
                    nc.tensor.matmul(
                        s_ps[:, 1, offc:],
                        kkt_sb[HD:P, ks],
                        qt_sb[HD:P, pair, qs],
                        start=True,
                        stop=True,
                        tile_position=(64, 0),
                    )
                    nc.scalar.activation(
                        p_sb[:, :, kt, offc:], s_ps[:, :, offc:], AF.Exp
                    )
                    if dk >= 0:  # diagonal block: causal triangle mask
                        nc.vector.tensor_mul(
                            p_sb[:, :, kt, off : off + P],
                            p_sb[:, :, kt, off : off + P],
                            tri[:],
                        )
                # --- phase B: PV accumulation, normalize, transpose ---
                for j in range(NT):
                    ktn = qb * 4 + j + 1  # kpos tiles 0..ktn-1
                    o_n = on_pool.tile([P, 2, HD], BF16, tag="on")
                    for half in range(2):
                        o_ps = pp.tile([P, HD + 1], F32, tag="O", bufs=O_BUFS)
                        for kt in range(ktn):
                            nc.tensor.matmul(
                                o_ps[:],
                                p_sb[:, half, kt, j * P : (j + 1) * P],
                                v1_sb[:, kt, :],
                                start=(kt == 0),
                                stop=(kt == ktn - 1),
                            )
                        rec = rc_pool.tile([P, 1], F32, tag="rec")
                        nc.vector.reciprocal(rec[:], o_ps[:, HD : HD + 1])
                        nc.vector.tensor_scalar_mul(
                            o_n[:, half, :], o_ps[:, 0:HD], rec[:]
                        )
                    nc.sync.dma_start_transpose(
                        ot_sb[:, pair, j * P : (j + 1) * P], o_n[:]
                    )
            # --- Wo partial for this q-block's 512 tokens ---
            for j in range(NT):
                rows = slice(qb * TB + j * P, qb * TB + (j + 1) * P)
                stg = stg_pool.tile([P, D], BF16, tag="stg")
                for nb in range(4):
                    wo_ps = pp.tile([P, TB], F32, tag="W", bufs=W_BUFS)
                    for ko in range(2):
                        nc.tensor.matmul(
                            wo_ps[:],
                            ot_sb[:, ko, j * P : (j + 1) * P],
                            wo_sb[:, ko, nb * TB : (nb + 1) * TB],
                            start=(ko == 0),
                            stop=(ko == 1),
                        )
                    nc.vector.tensor_copy(stg[:, nb * TB : (nb + 1) * TB], wo_ps[:])
                nc.sync.dma_start(out[b, rows, :], stg[:])

        for tb in range(NT):
            proj(tb)
        for qb in range(NT):
            attn(qb)


_NC_CACHE = {}


def get_nc():
    if "nc" not in _NC_CACHE:
        nc = bacc.Bacc("TRN2", target_bir_lowering=False, debug=False)
        with tile.TileContext(nc) as tc, ExitStack() as ctx:
            build_kernel(ctx, tc)
        nc.compile()
        _NC_CACHE["nc"] = nc
    return _NC_CACHE["nc"]


def make_in_maps(x, Wq, Wk, Wv, Wo):
    xT = np.ascontiguousarray(
        np.transpose(np.asarray(x, np.float32), (0, 2, 1))
    ).astype(ml_dtypes.bfloat16)
    Wq, Wk, Wv, Wo = (np.asarray(w, np.float32) for w in (Wq, Wk, Wv, Wo))
    in_maps = []
    for g in range(NCORES):
        in_maps.append(
            {
                "xT": xT,
                "wq": np.ascontiguousarray(
                    Wq[:, g * DQ : (g + 1) * DQ]
                ).astype(ml_dtypes.bfloat16),
                "wkv": np.ascontiguousarray(
                    np.concatenate(
                        [Wk[:, g * HD : (g + 1) * HD], Wv[:, g * HD : (g + 1) * HD]],
                        axis=1,
                    )
                ).astype(ml_dtypes.bfloat16),
                "wo": np.ascontiguousarray(
                    Wo[g * DQ : (g + 1) * DQ, :]
                ).astype(ml_dtypes.bfloat16),
            }
        )
    return in_maps


def run(x, Wq, Wk, Wv, Wo, trace=False):
    nc = get_nc()
    in_maps = make_in_maps(x, Wq, Wk, Wv, Wo)
    res = run_bass_kernel_spmd(nc, in_maps, list(range(NCORES)), trace=trace)
    acc = np.zeros((B, T, D), np.float32)
    for r in res.results:
        acc += np.asarray(r["out"], dtype=np.float32)
    return acc, res


def kernel(x, Wq, Wk, Wv, Wo):
    return run(x, Wq, Wk, Wv, Wo)[0]


# revision 5
# speedup vs baseline: 1.0943x; 1.0250x over previous
"""GQA attention kernel for Trainium2, tensor-parallel across 8 NeuronCores.

Problem: B=2, T=2048, D=2048, H=32 q-heads, G=8 kv-heads (GQA, rep=4), hd=64,
causal softmax attention + output projection, fp32 I/O.

Sharding (one KV group per core):
  core g: Wq[:, g*256:(g+1)*256], Wk/Wv[:, g*64:(g+1)*64], Wo[g*256:(g+1)*256, :]
  Each core computes its 4 heads' attention + partial output projection;
  host sums the 8 partial outputs (row-parallel Wo => partial-sum unshard).
  Partial outputs are stored bf16 (halves store bandwidth); host sums in f32.

Per-core dataflow (PE cost model charges out_free_size x cycles_per_row per
matmul; bf16 = 1.0 c/r at any free size, fp8 DoubleRow = 0.5 c/r):
  Projections: fp8 error-split DoubleRow - host supplies x and (64x-scaled)
  Wq/Wkv as fp8e4 hi+lo pairs; each projection chain accumulates three
  DoubleRow passes (hi@hi + lo@hi + hi@lo) over ko-pairs, matching bf16
  accuracy at half the PE cost. Three sequential 1-bank chains (Q pair0,
  Q pair1, KV) in a dedicated psum tag; 1/64 unscaling folded into the
  psum->SBUF copies (on DVE, keeping ACT free for exps).
  K duplicated to partitions 64..127 (SBUF-SBUF DMA); V transposed to
  row-major via PE identity matmuls -> V1 [kpos, 16, hd|1] with a ones col.
  Scores per (kt, pair): two bf16 matmuls (head halves, tile_position
  quadrants) into a 2-bank psum [128, 2, 512]; ONE exp per (kt, pair) ->
  P sbuf bf16. Diagonal kt trimmed to columns >= dk*128; causal triangle
  masked by multiplying with a precomputed upper-tri bf16 mask (DVE).
  PV: per (pair, half, j): out [q=128, hd|1=65] psum accumulated over kt
  with P slices as the stationary operand (65 cycles/matmul instead of 512).
  Normalize: DVE reciprocal of col 64 ([128,1]) * cols 0..63 -> o_n bf16;
  o_n [128q, 2*64] transposed to ot [128 dq, 128 q] by DMA-engine xbar.
  Wo partial: ot.T @ wo (bf16) -> psum [128, 512] x 4 -> stg bf16
  [128, 2048] -> one DMA per 128-token row block (the last batch defers its
  final two Wo blocks into the exp-bound tail and splits the last stores).
"""

import os
import sys

import numpy as np

for _p in ("/opt/trn_rl_repo", "/root/.axon_site/_ro/trn_rl_repo"):
    if os.path.isdir(_p) and _p not in sys.path:
        sys.path.insert(0, _p)

import ml_dtypes  # noqa: E402

import concourse.bass as bass  # noqa: E402
import concourse.mybir as mybir  # noqa: E402
import concourse.tile as tile  # noqa: E402
from concourse import bacc  # noqa: E402
from concourse.bass_utils import run_bass_kernel_spmd  # noqa: E402
from concourse.masks import make_identity  # noqa: E402
from contextlib import ExitStack  # noqa: E402

B, T, D = 2, 2048, 2048
G, REP, HD = 8, 4, 64
DQ = REP * HD  # 256 q-dims per core
NCORES = 8
P = 128
TB = 512  # q/t block size
KO = D // P  # 16 contraction subtiles for projections
KQ = 4  # ko tiles per x DMA load
NT = T // TB  # 4 t-blocks
NKT = T // P  # 16 kpos tiles
F32 = mybir.dt.float32
F32R = mybir.dt.float32r
BF16 = mybir.dt.bfloat16
FP8 = mybir.dt.float8e4
DR = mybir.MatmulPerfMode.DoubleRow
WSCALE = 64.0  # host multiplies weights by this before fp8 split
AF = mybir.ActivationFunctionType
SCALE = 1.0 / 8.0  # 1/sqrt(HD)
PJ_BUFS = 1
S_BUFS = 2
O_BUFS = 2
W_BUFS = 1


def build_kernel(ctx, tc):
    nc = tc.nc
    xh = nc.dram_tensor("xh", [B, D, T], FP8, kind="ExternalInput").ap()
    xl = nc.dram_tensor("xl", [B, D, T], FP8, kind="ExternalInput").ap()
    wqh = nc.dram_tensor("wqh", [D, DQ], FP8, kind="ExternalInput").ap()
    wql = nc.dram_tensor("wql", [D, DQ], FP8, kind="ExternalInput").ap()
    wkvh = nc.dram_tensor("wkvh", [D, 2 * HD], FP8, kind="ExternalInput").ap()
    wkvl = nc.dram_tensor("wkvl", [D, 2 * HD], FP8, kind="ExternalInput").ap()
    wo = nc.dram_tensor("wo", [DQ, D], BF16, kind="ExternalInput").ap()
    out = nc.dram_tensor("out", [B, T, D], BF16, kind="ExternalOutput").ap()

    wpool = ctx.enter_context(tc.tile_pool(name="w", bufs=1))
    qt_pool = ctx.enter_context(tc.tile_pool(name="qt", bufs=2))
    kkt_pool = ctx.enter_context(tc.tile_pool(name="kkt", bufs=2))
    vt_pool = ctx.enter_context(tc.tile_pool(name="vt", bufs=2))
    v_pool = ctx.enter_context(tc.tile_pool(name="v", bufs=2))
    xt_pool = ctx.enter_context(tc.tile_pool(name="xt", bufs=5))
    p_pool = ctx.enter_context(tc.tile_pool(name="p", bufs=2))
    on_pool = ctx.enter_context(tc.tile_pool(name="on", bufs=3))
    rc_pool = ctx.enter_context(tc.tile_pool(name="rc", bufs=3))
    ot_pool = ctx.enter_context(tc.tile_pool(name="ot", bufs=2))
    stg_pool = ctx.enter_context(tc.tile_pool(name="stg", bufs=2))
    pp = ctx.enter_context(tc.tile_pool(name="pp", bufs=2, space="PSUM"))

    # persistent weights (SP/HWDGE queue; Pool is reserved for xt loads).
    # wq/wkv split into ko-chunks so the first matmuls wait only on chunk 0.
    wqh_sb = wpool.tile([P, KO, DQ], FP8, tag="wqh")
    wql_sb = wpool.tile([P, KO, DQ], FP8, tag="wql")
    wkvh_sb = wpool.tile([P, KO, 2 * HD], FP8, tag="wkvh")
    wkvl_sb = wpool.tile([P, KO, 2 * HD], FP8, tag="wkvl")
    for sb, dr in ((wqh_sb, wqh), (wkvh_sb, wkvh), (wql_sb, wql), (wkvl_sb, wkvl)):
        r = dr.rearrange("(ko p) m -> p ko m", p=P)
        for c in range(0, KO, KQ):
            nc.sync.dma_start(sb[:, c : c + KQ, :], r[:, c : c + KQ, :])
    wo_sb = wpool.tile([P, DQ // P, D], BF16, tag="wo")
    nc.sync.dma_start(wo_sb[:], wo.rearrange("(ko p) m -> p ko m", p=P))
    # upper-triangular causal mask (keep f >= p), two identical copies so one
    # tensor_tensor covers both head halves of a pair at once
    ident = wpool.tile([HD, HD], BF16, tag="ident")
    make_identity(nc, ident[:])
    tri = wpool.tile([P, 2, P], BF16, tag="tri")
    nc.gpsimd.memset(tri[:], 1.0)
    for h in range(2):
        nc.gpsimd.affine_select(
            out=tri[:, h, :],
            in_=tri[:, h, :],
            compare_op=mybir.AluOpType.is_ge,
            fill=0.0,
            base=0,
            channel_multiplier=-1,
            pattern=[[1, P]],
        )

    for b in range(B):
        qt_sb = qt_pool.tile([P, 2, T], BF16, tag="qt")  # QT, scaled by 1/8
        kkt_sb = kkt_pool.tile([P, T], BF16, tag="kkt")  # KT duplicated twice
        vt_sb = vt_pool.tile([HD, T], BF16, tag="vt")  # VT bf16 on 64 parts
        v1_sb = v_pool.tile([P, NKT, HD + 1], BF16, tag="v1")
        nc.gpsimd.memset(v1_sb[:, :, HD : HD + 1], 1.0)

        def proj(tb):
            # ---------------- projections for t-block tb ----------------
            ts = slice(tb * TB, (tb + 1) * TB)
            xhs, xls = [], []
            for src_t, lst, tag in ((xh, xhs, "xh"), (xl, xls, "xl")):
                for kq in range(KO // KQ):
                    xt = xt_pool.tile([P, KQ, TB], FP8, tag=tag)
                    nc.gpsimd.dma_start(
                        xt[:],
                        src_t[b, kq * KQ * P : (kq + 1) * KQ * P, ts].rearrange(
                            "(q p) t -> p q t", p=P
                        ),
                    )
                    lst.append(xt)
            # three sequential 1-bank accumulation chains (pair0, pair1, kv),
            # each as 3 fp8 DoubleRow passes (hi@hi + lo@hi + hi@lo) over
            # ko-pairs: error-split fp8 at 0.5 cycles/row beats bf16 on both
            # speed and accuracy. Weights are host-scaled by WSCALE so their
            # lo residuals stay in fp8 normal range; psum is WSCALE too big.
            for ci, (whsb, wlsb, lo) in enumerate(
                ((wqh_sb, wql_sb, 0), (wqh_sb, wql_sb, P), (wkvh_sb, wkvl_sb, 0))
            ):
                c_ps = pp.tile([P, TB], F32, tag="PJ", bufs=PJ_BUFS)
                passes = ((whsb, xhs), (whsb, xls), (wlsb, xhs))
                n_mm = len(passes) * (KO // 2)
                i = 0
                for wsb, xlist in passes:
                    for kp in range(KO // 2):
                        ko = 2 * kp
                        nc.tensor.matmul(
                            c_ps[:],
                            wsb[:, ko : ko + 2, lo : lo + P],
                            xlist[ko // KQ][:, ko % KQ : ko % KQ + 2, :],
                            start=(i == 0),
                            stop=(i == n_mm - 1),
                            perf_mode=DR,
                        )
                        i += 1
                if ci < 2:
                    # on DVE, not ACT: keeps the exp queue free of copies
                    nc.vector.tensor_scalar_mul(
                        qt_sb[:, ci, ts], c_ps[:], SCALE / WSCALE
                    )
                else:
                    nc.vector.tensor_scalar_mul(
                        kkt_sb[0:HD, ts], c_ps[0:HD, :], 1.0 / WSCALE
                    )
                    nc.vector.tensor_scalar_mul(
                        vt_sb[:, ts], c_ps[HD:P, :], 1.0 / WSCALE
                    )
            # duplicate KT to partitions 64..127 (SBUF->SBUF DMA moves partitions)
            nc.sync.dma_start(kkt_sb[HD:P, ts], kkt_sb[0:HD, ts])
            # V transpose via PE identity matmul: [64, 128] -> [128, 64]
            # (the [64,128] xbar DMA-transpose corrupts data on HW; the
            # [128,128] o_n xbar transpose below is fine)
            for kt in range(4 * tb, 4 * tb + 4):
                tr_ps = pp.tile([P, HD], BF16, tag="W", bufs=W_BUFS)
                nc.tensor.transpose(
                    tr_ps[:], vt_sb[:, kt * P : (kt + 1) * P], ident[:]
                )
                nc.vector.tensor_copy(v1_sb[:, kt, 0:HD], tr_ps[:])

        def attn(qb):
            # ------------- attention + output proj for q-block qb -----------
            nkt = 4 * (qb + 1)  # causal: kpos tiles 0..nkt-1
            # --- phase A: scores + exp, kt-major / pair-minor: two
            # independent score->exp streams keep both S slots busy ---
            p4 = p_pool.tile([P, 2, 2, nkt, TB], BF16,
                             tag=f"P{qb % 2}", bufs=1)
            for kt in range(nkt):
                for pair in range(2):
                    p_sb = p4[:, pair]
                    ks = slice(kt * P, (kt + 1) * P)
                    dk = kt - qb * 4
                    off = max(dk, 0) * P  # first potentially-valid column
                    offc = off  # computed column start (bf16: any free size ok)
                    s_ps = pp.tile([P, 2, TB], F32, tag="S", bufs=S_BUFS)
                    qs = slice(qb * TB + offc, (qb + 1) * TB)
                    nc.tensor.matmul(
                        s_ps[:, 0, offc:],
                        kkt_sb[0:HD, ks],
                        qt_sb[0:HD, pair, qs],
                        start=True,
                        stop=True,
                        tile_position=(0, 0),
                    )
                    nc.tensor.matmul(
                        s_ps[:, 1, offc:],
                        kkt_sb[HD:P, ks],
                        qt_sb[HD:P, pair, qs],
                        start=True,
                        stop=True,
                        tile_position=(64, 0),
                    )
                    nc.scalar.activation(
                        p_sb[:, :, kt, offc:], s_ps[:, :, offc:], AF.Exp
                    )
                    if dk >= 0:  # diagonal block: causal triangle mask
                        nc.vector.tensor_mul(
                            p_sb[:, :, kt, off : off + P],
                            p_sb[:, :, kt, off : off + P],
                            tri[:],
                        )
            return p4

        def attn_b(qb, p4):
            nkt = 4 * (qb + 1)
            ot_sb = ot_pool.tile([P, 2, TB], BF16, tag="ot")
            for pair in range(2):
                p_sb = p4[:, pair]
                # --- phase B: PV accumulation, normalize, transpose ---
                for j in range(NT):
                    ktn = qb * 4 + j + 1  # kpos tiles 0..ktn-1
                    o_n = on_pool.tile([P, 2, HD], BF16, tag="on")
                    for half in range(2):
                        o_ps = pp.tile([P, HD + 1], F32, tag="O", bufs=O_BUFS)
                        for kt in range(ktn):
                            nc.tensor.matmul(
                                o_ps[:],
                                p_sb[:, half, kt, j * P : (j + 1) * P],
                                v1_sb[:, kt, :],
                                start=(kt == 0),
                                stop=(kt == ktn - 1),
                            )
                        rec = rc_pool.tile([P, 1], F32, tag="rec")
                        nc.vector.reciprocal(rec[:], o_ps[:, HD : HD + 1])
                        nc.vector.tensor_scalar_mul(
                            o_n[:, half, :], o_ps[:, 0:HD], rec[:]
                        )
                    nc.sync.dma_start_transpose(
                        ot_sb[:, pair, j * P : (j + 1) * P], o_n[:]
                    )
            return ot_sb

        def wo_block(qb, ot_sb, split_stores=False):
            # --- Wo partial for this q-block's 512 tokens ---
            for j in range(NT):
                rows = slice(qb * TB + j * P, qb * TB + (j + 1) * P)
                stg = stg_pool.tile([P, D], BF16, tag="stg")
                for nb in range(4):
                    wo_ps = pp.tile([P, TB], F32, tag="W", bufs=W_BUFS)
                    for ko in range(2):
                        nc.tensor.matmul(
                            wo_ps[:],
                            ot_sb[:, ko, j * P : (j + 1) * P],
                            wo_sb[:, ko, nb * TB : (nb + 1) * TB],
                            start=(ko == 0),
                            stop=(ko == 1),
                        )
                    nc.vector.tensor_copy(stg[:, nb * TB : (nb + 1) * TB], wo_ps[:])
                    if split_stores:
                        nc.sync.dma_start(
                            out[b, rows, nb * TB : (nb + 1) * TB],
                            stg[:, nb * TB : (nb + 1) * TB],
                        )
                if not split_stores:
                    nc.sync.dma_start(out[b, rows, :], stg[:])

        for tb in range(NT):
            proj(tb)
        # Phase A emitted one q-block ahead of phase B: the next block's
        # scores/exps outrank the previous block's PV/Wo in scheduler
        # priority, keeping the serial exp stream (the attention-phase
        # bottleneck) continuously fed.
        p1 = attn(0)
        p2 = attn(1)
        wo_block(0, attn_b(0, p1))
        p3 = attn(2)
        wo_block(1, attn_b(1, p2))
        p4_ = attn(3)
        wo_block(2, attn_b(2, p3))
        wo_block(3, attn_b(3, p4_), split_stores=(b == B - 1))


_NC_CACHE = {}


def get_nc():
    if "nc" not in _NC_CACHE:
        nc = bacc.Bacc("TRN2", target_bir_lowering=False, debug=False)
        with tile.TileContext(nc) as tc, ExitStack() as ctx:
            build_kernel(ctx, tc)
        nc.compile()
        _NC_CACHE["nc"] = nc
    return _NC_CACHE["nc"]


def make_in_maps(x, Wq, Wk, Wv, Wo):
    FP8NP = ml_dtypes.float8_e4m3

    def fp8_split(a):
        hi = a.astype(FP8NP)
        lo = (a - hi.astype(np.float32)).astype(FP8NP)
        return hi, lo

    xT = np.ascontiguousarray(np.transpose(np.asarray(x, np.float32), (0, 2, 1)))
    xh, xl = fp8_split(xT)
    Wq, Wk, Wv, Wo = (np.asarray(w, np.float32) for w in (Wq, Wk, Wv, Wo))
    in_maps = []
    for g in range(NCORES):
        in_maps.append(
            {
                "xh": xh,
                "xl": xl,
                **dict(
                    zip(
                        ("wqh", "wql"),
                        fp8_split(
                            64.0 * np.ascontiguousarray(Wq[:, g * DQ : (g + 1) * DQ])
                        ),
                    )
                ),
                **dict(
                    zip(
                        ("wkvh", "wkvl"),
                        fp8_split(
                            64.0
                            * np.ascontiguousarray(
                                np.concatenate(
                                    [
                                        Wk[:, g * HD : (g + 1) * HD],
                                        Wv[:, g * HD : (g + 1) * HD],
                                    ],
                                    axis=1,
                                )
                            )
                        ),
                    )
                ),
                "wo": np.ascontiguousarray(
                    Wo[g * DQ : (g + 1) * DQ, :]
                ).astype(ml_dtypes.bfloat16),
            }
        )
    return in_maps


def run(x, Wq, Wk, Wv, Wo, trace=False):
    nc = get_nc()
    in_maps = make_in_maps(x, Wq, Wk, Wv, Wo)
    res = run_bass_kernel_spmd(nc, in_maps, list(range(NCORES)), trace=trace)
    acc = np.zeros((B, T, D), np.float32)
    for r in res.results:
        acc += np.asarray(r["out"], dtype=np.float32)
    return acc, res


def kernel(x, Wq, Wk, Wv, Wo):
    return run(x, Wq, Wk, Wv, Wo)[0]


# revision 6
# speedup vs baseline: 1.1064x; 1.0111x over previous
"""GQA attention kernel for Trainium2, tensor-parallel across 8 NeuronCores.

Problem: B=2, T=2048, D=2048, H=32 q-heads, G=8 kv-heads (GQA, rep=4), hd=64,
causal softmax attention + output projection, fp32 I/O.

Sharding (one KV group per core):
  core g: Wq[:, g*256:(g+1)*256], Wk/Wv[:, g*64:(g+1)*64], Wo[g*256:(g+1)*256, :]
  Each core computes its 4 heads' attention + partial output projection;
  host sums the 8 partial outputs (row-parallel Wo => partial-sum unshard).
  Partial outputs are stored bf16 (halves store bandwidth); host sums in f32.

Per-core dataflow (PE cost model charges out_free_size x cycles_per_row per
matmul; bf16 = 1.0 c/r at any free size, fp8 DoubleRow = 0.5 c/r):
  Projections: fp8 error-split DoubleRow - host supplies x and (64x-scaled)
  Wq/Wkv as fp8e4 hi+lo pairs; each projection chain accumulates three
  DoubleRow passes (hi@hi + lo@hi + hi@lo) over ko-pairs, matching bf16
  accuracy at half the PE cost. Three sequential 1-bank chains (Q pair0,
  Q pair1, KV) in a dedicated psum tag; 1/64 unscaling folded into the
  psum->SBUF copies (on DVE, keeping ACT free for exps).
  K duplicated to partitions 64..127 (SBUF-SBUF DMA); V transposed to
  row-major via PE identity matmuls -> V1 [kpos, 16, hd|1] with a ones col.
  Scores per (kt, pair): two bf16 matmuls (head halves, tile_position
  quadrants) into a 2-bank psum [128, 2, 512]; ONE exp per (kt, pair) ->
  P sbuf bf16. Diagonal kt trimmed to columns >= dk*128; causal triangle
  masked by multiplying with a precomputed upper-tri bf16 mask (DVE).
  PV: per (pair, half, j): out [q=128, hd|1=65] psum accumulated over kt
  with P slices as the stationary operand (65 cycles/matmul instead of 512).
  Normalize: DVE reciprocal of col 64 ([128,1]) * cols 0..63 -> o_n bf16;
  o_n [128q, 2*64] transposed to ot [128 dq, 128 q] by DMA-engine xbar.
  Wo partial: ot.T @ wo (bf16) -> psum [128, 512] x 4 -> stg bf16
  [128, 2048] -> one DMA per 128-token row block (the last batch defers its
  final two Wo blocks into the exp-bound tail and splits the last stores).
"""

import os
import sys

import numpy as np

for _p in ("/opt/trn_rl_repo", "/root/.axon_site/_ro/trn_rl_repo"):
    if os.path.isdir(_p) and _p not in sys.path:
        sys.path.insert(0, _p)

import ml_dtypes  # noqa: E402

import concourse.bass as bass  # noqa: E402
import concourse.mybir as mybir  # noqa: E402
import concourse.tile as tile  # noqa: E402
from concourse import bacc  # noqa: E402
from concourse.bass_utils import run_bass_kernel_spmd  # noqa: E402
from concourse.masks import make_identity  # noqa: E402
from contextlib import ExitStack  # noqa: E402

B, T, D = 2, 2048, 2048
G, REP, HD = 8, 4, 64
DQ = REP * HD  # 256 q-dims per core
NCORES = 8
P = 128
TB = 512  # q/t block size
KO = D // P  # 16 contraction subtiles for projections
KQ = 4  # ko tiles per x DMA load
NT = T // TB  # 4 t-blocks
NKT = T // P  # 16 kpos tiles
F32 = mybir.dt.float32
F32R = mybir.dt.float32r
BF16 = mybir.dt.bfloat16
FP8 = mybir.dt.float8e4
DR = mybir.MatmulPerfMode.DoubleRow
WSCALE = 64.0  # host multiplies weights by this before fp8 split
AF = mybir.ActivationFunctionType
SCALE = 1.0 / 8.0  # 1/sqrt(HD)
PJ_BUFS = 1
S_BUFS = 2
O_BUFS = 2
W_BUFS = 1


def build_kernel(ctx, tc):
    nc = tc.nc
    xh = nc.dram_tensor("xh", [B, D, T], FP8, kind="ExternalInput").ap()
    xl = nc.dram_tensor("xl", [B, D, T], FP8, kind="ExternalInput").ap()
    wqh = nc.dram_tensor("wqh", [D, DQ], FP8, kind="ExternalInput").ap()
    wql = nc.dram_tensor("wql", [D, DQ], FP8, kind="ExternalInput").ap()
    wkvh = nc.dram_tensor("wkvh", [D, 2 * HD], FP8, kind="ExternalInput").ap()
    wkvl = nc.dram_tensor("wkvl", [D, 2 * HD], FP8, kind="ExternalInput").ap()
    wo = nc.dram_tensor("wo", [DQ, D], BF16, kind="ExternalInput").ap()
    out = nc.dram_tensor("out", [B, T, D], BF16, kind="ExternalOutput").ap()

    wpool = ctx.enter_context(tc.tile_pool(name="w", bufs=1))
    qt_pool = ctx.enter_context(tc.tile_pool(name="qt", bufs=2))
    kkt_pool = ctx.enter_context(tc.tile_pool(name="kkt", bufs=2))
    vt_pool = ctx.enter_context(tc.tile_pool(name="vt", bufs=2))
    v_pool = ctx.enter_context(tc.tile_pool(name="v", bufs=2))
    xt_pool = ctx.enter_context(tc.tile_pool(name="xt", bufs=5))
    p_pool = ctx.enter_context(tc.tile_pool(name="p", bufs=2))
    on_pool = ctx.enter_context(tc.tile_pool(name="on", bufs=3))
    rc_pool = ctx.enter_context(tc.tile_pool(name="rc", bufs=3))
    ot_pool = ctx.enter_context(tc.tile_pool(name="ot", bufs=2))
    stg_pool = ctx.enter_context(tc.tile_pool(name="stg", bufs=2))
    pp = ctx.enter_context(tc.tile_pool(name="pp", bufs=2, space="PSUM"))

    # persistent weights (SP/HWDGE queue; Pool is reserved for xt loads).
    # wq/wkv split into ko-chunks so the first matmuls wait only on chunk 0.
    wqh_sb = wpool.tile([P, KO, DQ], FP8, tag="wqh")
    wql_sb = wpool.tile([P, KO, DQ], FP8, tag="wql")
    wkvh_sb = wpool.tile([P, KO, 2 * HD], FP8, tag="wkvh")
    wkvl_sb = wpool.tile([P, KO, 2 * HD], FP8, tag="wkvl")
    for sb, dr in ((wqh_sb, wqh), (wkvh_sb, wkvh), (wql_sb, wql), (wkvl_sb, wkvl)):
        r = dr.rearrange("(ko p) m -> p ko m", p=P)
        for c in range(0, KO, KQ):
            nc.sync.dma_start(sb[:, c : c + KQ, :], r[:, c : c + KQ, :])
    wo_sb = wpool.tile([P, DQ // P, D], BF16, tag="wo")
    nc.sync.dma_start(wo_sb[:], wo.rearrange("(ko p) m -> p ko m", p=P))
    # upper-triangular causal mask (keep f >= p), two identical copies so one
    # tensor_tensor covers both head halves of a pair at once
    ident = wpool.tile([HD, HD], BF16, tag="ident")
    make_identity(nc, ident[:])
    tri = wpool.tile([P, 2, P], BF16, tag="tri")
    nc.gpsimd.memset(tri[:], 1.0)
    for h in range(2):
        nc.gpsimd.affine_select(
            out=tri[:, h, :],
            in_=tri[:, h, :],
            compare_op=mybir.AluOpType.is_ge,
            fill=0.0,
            base=0,
            channel_multiplier=-1,
            pattern=[[1, P]],
        )

    for b in range(B):
        qt_sb = qt_pool.tile([P, 2, T], BF16, tag="qt")  # QT, scaled by 1/8
        kkt_sb = kkt_pool.tile([P, T], BF16, tag="kkt")  # KT duplicated twice
        vt_sb = vt_pool.tile([HD, T], BF16, tag="vt")  # VT bf16 on 64 parts
        v1_sb = v_pool.tile([P, NKT, HD + 1], BF16, tag="v1")
        nc.gpsimd.memset(v1_sb[:, :, HD : HD + 1], 1.0)

        def proj(tb):
            # ---------------- projections for t-block tb ----------------
            ts = slice(tb * TB, (tb + 1) * TB)
            xhs, xls = [], []
            for src_t, lst, tag in ((xh, xhs, "xh"), (xl, xls, "xl")):
                for kq in range(KO // KQ):
                    xt = xt_pool.tile([P, KQ, TB], FP8, tag=tag)
                    nc.gpsimd.dma_start(
                        xt[:],
                        src_t[b, kq * KQ * P : (kq + 1) * KQ * P, ts].rearrange(
                            "(q p) t -> p q t", p=P
                        ),
                    )
                    lst.append(xt)
            # three sequential 1-bank accumulation chains (pair0, pair1, kv),
            # each as 3 fp8 DoubleRow passes (hi@hi + lo@hi + hi@lo) over
            # ko-pairs: error-split fp8 at 0.5 cycles/row beats bf16 on both
            # speed and accuracy. Weights are host-scaled by WSCALE so their
            # lo residuals stay in fp8 normal range; psum is WSCALE too big.
            for ci, (whsb, wlsb, lo) in enumerate(
                ((wqh_sb, wql_sb, 0), (wqh_sb, wql_sb, P), (wkvh_sb, wkvl_sb, 0))
            ):
                if b == 0 and tb == 0 and ci == 2:
                    c_ps = pp.tile([P, TB], F32, tag="O", bufs=O_BUFS)
                elif b == 0 and tb >= 1 and ci == 1:
                    c_ps = pp.tile([P, TB], F32, tag="W", bufs=W_BUFS)
                elif b == 0 and tb == 0 and ci < 2:
                    # before any attention exists the score psum is idle:
                    # borrow S slots so the first three chains overlap
                    sbig = pp.tile(
                        [P, 2, TB], F32, tag="S", bufs=S_BUFS, name=f"sb{ci}"
                    )
                    c_ps = sbig[:, 0]
                else:
                    c_ps = pp.tile([P, TB], F32, tag="PJ", bufs=PJ_BUFS)
                passes = ((whsb, xhs), (whsb, xls), (wlsb, xhs))
                n_mm = len(passes) * (KO // 2)
                i = 0
                for wsb, xlist in passes:
                    for kp in range(KO // 2):
                        ko = 2 * kp
                        nc.tensor.matmul(
                            c_ps[:],
                            wsb[:, ko : ko + 2, lo : lo + P],
                            xlist[ko // KQ][:, ko % KQ : ko % KQ + 2, :],
                            start=(i == 0),
                            stop=(i == n_mm - 1),
                            perf_mode=DR,
                        )
                        i += 1
                if ci < 2:
                    # on DVE, not ACT: keeps the exp queue free of copies
                    nc.vector.tensor_scalar_mul(
                        qt_sb[:, ci, ts], c_ps[:], SCALE / WSCALE
                    )
                else:
                    nc.vector.tensor_scalar_mul(
                        kkt_sb[0:HD, ts], c_ps[0:HD, :], 1.0 / WSCALE
                    )
                    nc.vector.tensor_scalar_mul(
                        vt_sb[:, ts], c_ps[HD:P, :], 1.0 / WSCALE
                    )
            # duplicate KT to partitions 64..127 (SBUF->SBUF DMA moves partitions)
            nc.sync.dma_start(kkt_sb[HD:P, ts], kkt_sb[0:HD, ts])
            # V transpose via PE identity matmul: [64, 128] -> [128, 64]
            # (the [64,128] xbar DMA-transpose corrupts data on HW; the
            # [128,128] o_n xbar transpose below is fine)
            for kt in range(4 * tb, 4 * tb + 4):
                tr_ps = pp.tile([P, HD], BF16, tag="W", bufs=W_BUFS)
                nc.tensor.transpose(
                    tr_ps[:], vt_sb[:, kt * P : (kt + 1) * P], ident[:]
                )
                nc.vector.tensor_copy(v1_sb[:, kt, 0:HD], tr_ps[:])

        def attn(qb):
            # ------------- attention + output proj for q-block qb -----------
            nkt = 4 * (qb + 1)  # causal: kpos tiles 0..nkt-1
            # --- phase A: scores + exp, kt-major / pair-minor: two
            # independent score->exp streams keep both S slots busy ---
            p4 = p_pool.tile([P, 2, 2, nkt, TB], BF16,
                             tag=f"P{qb % 2}", bufs=1)
            for kt in range(nkt):
                for pair in range(2):
                    p_sb = p4[:, pair]
                    ks = slice(kt * P, (kt + 1) * P)
                    dk = kt - qb * 4
                    off = max(dk, 0) * P  # first potentially-valid column
                    offc = off  # computed column start (bf16: any free size ok)
                    s_ps = pp.tile([P, 2, TB], F32, tag="S", bufs=S_BUFS)
                    qs = slice(qb * TB + offc, (qb + 1) * TB)
                    nc.tensor.matmul(
                        s_ps[:, 0, offc:],
                        kkt_sb[0:HD, ks],
                        qt_sb[0:HD, pair, qs],
                        start=True,
                        stop=True,
                        tile_position=(0, 0),
                    )
                    nc.tensor.matmul(
                        s_ps[:, 1, offc:],
                        kkt_sb[HD:P, ks],
                        qt_sb[HD:P, pair, qs],
                        start=True,
                        stop=True,
                        tile_position=(64, 0),
                    )
                    nc.scalar.activation(
                        p_sb[:, :, kt, offc:], s_ps[:, :, offc:], AF.Exp
                    )
                    if dk >= 0:  # diagonal block: causal triangle mask
                        nc.vector.tensor_mul(
                            p_sb[:, :, kt, off : off + P],
                            p_sb[:, :, kt, off : off + P],
                            tri[:],
                        )
            return p4

        def attn_b(qb, p4):
            nkt = 4 * (qb + 1)
            ot_sb = ot_pool.tile([P, 2, TB], BF16, tag="ot")
            for pair in range(2):
                p_sb = p4[:, pair]
                # --- phase B: PV accumulation, normalize, transpose ---
                for j in range(NT):
                    ktn = qb * 4 + j + 1  # kpos tiles 0..ktn-1
                    o_n = on_pool.tile([P, 2, HD], BF16, tag="on")
                    for half in range(2):
                        o_ps = pp.tile([P, HD + 1], F32, tag="O", bufs=O_BUFS)
                        for kt in range(ktn):
                            nc.tensor.matmul(
                                o_ps[:],
                                p_sb[:, half, kt, j * P : (j + 1) * P],
                                v1_sb[:, kt, :],
                                start=(kt == 0),
                                stop=(kt == ktn - 1),
                            )
                        rec = rc_pool.tile([P, 1], F32, tag="rec")
                        nc.vector.reciprocal(rec[:], o_ps[:, HD : HD + 1])
                        nc.vector.tensor_scalar_mul(
                            o_n[:, half, :], o_ps[:, 0:HD], rec[:]
                        )
                    nc.sync.dma_start_transpose(
                        ot_sb[:, pair, j * P : (j + 1) * P], o_n[:]
                    )
            return ot_sb

        def wo_block(qb, ot_sb, split_stores=False):
            # --- Wo partial for this q-block's 512 tokens ---
            for j in range(NT):
                rows = slice(qb * TB + j * P, qb * TB + (j + 1) * P)
                stg = stg_pool.tile([P, D], BF16, tag="stg")
                for nb in range(4):
                    wo_ps = pp.tile([P, TB], F32, tag="W", bufs=W_BUFS)
                    for ko in range(2):
                        nc.tensor.matmul(
                            wo_ps[:],
                            ot_sb[:, ko, j * P : (j + 1) * P],
                            wo_sb[:, ko, nb * TB : (nb + 1) * TB],
                            start=(ko == 0),
                            stop=(ko == 1),
                        )
                    nc.vector.tensor_copy(stg[:, nb * TB : (nb + 1) * TB], wo_ps[:])
                    if split_stores:
                        nc.sync.dma_start(
                            out[b, rows, nb * TB : (nb + 1) * TB],
                            stg[:, nb * TB : (nb + 1) * TB],
                        )
                if not split_stores:
                    nc.sync.dma_start(out[b, rows, :], stg[:])

        for tb in range(NT):
            proj(tb)
        # Phase A emitted one q-block ahead of phase B: the next block's
        # scores/exps outrank the previous block's PV/Wo in scheduler
        # priority, keeping the serial exp stream (the attention-phase
        # bottleneck) continuously fed.
        p1 = attn(0)
        p2 = attn(1)
        wo_block(0, attn_b(0, p1))
        p3 = attn(2)
        wo_block(1, attn_b(1, p2))
        p4_ = attn(3)
        wo_block(2, attn_b(2, p3))
        wo_block(3, attn_b(3, p4_), split_stores=(b == B - 1))


_NC_CACHE = {}


def get_nc():
    if "nc" not in _NC_CACHE:
        nc = bacc.Bacc("TRN2", target_bir_lowering=False, debug=False)
        with tile.TileContext(nc) as tc, ExitStack() as ctx:
            build_kernel(ctx, tc)
        nc.compile()
        _NC_CACHE["nc"] = nc
    return _NC_CACHE["nc"]


def make_in_maps(x, Wq, Wk, Wv, Wo):
    FP8NP = ml_dtypes.float8_e4m3

    def fp8_split(a):
        hi = a.astype(FP8NP)
        lo = (a - hi.astype(np.float32)).astype(FP8NP)
        return hi, lo

    xT = np.ascontiguousarray(np.transpose(np.asarray(x, np.float32), (0, 2, 1)))
    xh, xl = fp8_split(xT)
    Wq, Wk, Wv, Wo = (np.asarray(w, np.float32) for w in (Wq, Wk, Wv, Wo))
    in_maps = []
    for g in range(NCORES):
        in_maps.append(
            {
                "xh": xh,
                "xl": xl,
                **dict(
                    zip(
                        ("wqh", "wql"),
                        fp8_split(
                            64.0 * np.ascontiguousarray(Wq[:, g * DQ : (g + 1) * DQ])
                        ),
                    )
                ),
                **dict(
                    zip(
                        ("wkvh", "wkvl"),
                        fp8_split(
                            64.0
                            * np.ascontiguousarray(
                                np.concatenate(
                                    [
                                        Wk[:, g * HD : (g + 1) * HD],
                                        Wv[:, g * HD : (g + 1) * HD],
                                    ],
                                    axis=1,
                                )
                            )
                        ),
                    )
                ),
                "wo": np.ascontiguousarray(
                    Wo[g * DQ : (g + 1) * DQ, :]
                ).astype(ml_dtypes.bfloat16),
            }
        )
    return in_maps


def run(x, Wq, Wk, Wv, Wo, trace=False):
    nc = get_nc()
    in_maps = make_in_maps(x, Wq, Wk, Wv, Wo)
    res = run_bass_kernel_spmd(nc, in_maps, list(range(NCORES)), trace=trace)
    acc = np.zeros((B, T, D), np.float32)
    for r in res.results:
        acc += np.asarray(r["out"], dtype=np.float32)
    return acc, res


def kernel(x, Wq, Wk, Wv, Wo):
    return run(x, Wq, Wk, Wv, Wo)[0]


# revision 7
# speedup vs baseline: 1.1345x; 1.0254x over previous
"""GQA attention kernel for Trainium2, tensor-parallel across 8 NeuronCores.

Problem: B=2, T=2048, D=2048, H=32 q-heads, G=8 kv-heads (GQA, rep=4), hd=64,
causal softmax attention + output projection, fp32 I/O.

Sharding (one KV group per core):
  core g: Wq[:, g*256:(g+1)*256], Wk/Wv[:, g*64:(g+1)*64], Wo[g*256:(g+1)*256, :]
  Each core computes its 4 heads' attention + partial output projection;
  host sums the 8 partial outputs (row-parallel Wo => partial-sum unshard).
  Partial outputs are stored bf16 (halves store bandwidth); host sums in f32.

Per-core dataflow (PE cost model charges out_free_size x cycles_per_row per
matmul; bf16 = 1.0 c/r at any free size, fp8 DoubleRow = 0.5 c/r):
  Projections: fp8 error-split DoubleRow - host supplies x and (64x-scaled)
  Wq/Wkv as fp8e4 hi+lo pairs; each projection chain accumulates three
  DoubleRow passes (hi@hi + lo@hi + hi@lo) over ko-pairs, matching bf16
  accuracy at half the PE cost. Three sequential 1-bank chains (Q pair0,
  Q pair1, KV) in a dedicated psum tag; 1/64 unscaling folded into the
  psum->SBUF copies (on DVE, keeping ACT free for exps).
  K duplicated to partitions 64..127 (SBUF-SBUF DMA); V transposed to
  row-major via PE identity matmuls -> V1 [kpos, 16, hd|1] with a ones col.
  Scores per (kt, pair): two bf16 matmuls (head halves, tile_position
  quadrants) into a 2-bank psum [128, 2, 512]; ONE exp per (kt, pair) ->
  P sbuf bf16. Diagonal kt trimmed to columns >= dk*128; causal triangle
  masked by multiplying with a precomputed upper-tri bf16 mask (DVE).
  PV: per (pair, half, j): out [q=128, hd|1=65] psum accumulated over kt
  with P slices as the stationary operand (65 cycles/matmul instead of 512).
  Normalize: DVE reciprocal of col 64 ([128,1]) * cols 0..63 -> o_n bf16;
  o_n [128q, 2*64] transposed to ot [128 dq, 128 q] by DMA-engine xbar.
  Wo partial: ot.T @ wo (bf16) -> psum [128, 512] x 4 -> stg bf16
  [128, 2048] -> one DMA per 128-token row block (the last batch defers its
  final two Wo blocks into the exp-bound tail and splits the last stores).
"""

import os
import sys

import numpy as np

for _p in ("/opt/trn_rl_repo", "/root/.axon_site/_ro/trn_rl_repo"):
    if os.path.isdir(_p) and _p not in sys.path:
        sys.path.insert(0, _p)

import ml_dtypes  # noqa: E402

import concourse.bass as bass  # noqa: E402
import concourse.mybir as mybir  # noqa: E402
import concourse.tile as tile  # noqa: E402
from concourse import bacc  # noqa: E402
from concourse.bass_utils import run_bass_kernel_spmd  # noqa: E402
from concourse.masks import make_identity  # noqa: E402
from contextlib import ExitStack  # noqa: E402

B, T, D = 2, 2048, 2048
G, REP, HD = 8, 4, 64
DQ = REP * HD  # 256 q-dims per core
NCORES = 8
P = 128
TB = 512  # q/t block size
KO = D // P  # 16 contraction subtiles for projections
KQ = 4  # ko tiles per x DMA load
NT = T // TB  # 4 t-blocks
NKT = T // P  # 16 kpos tiles
F32 = mybir.dt.float32
F32R = mybir.dt.float32r
BF16 = mybir.dt.bfloat16
FP8 = mybir.dt.float8e4
DR = mybir.MatmulPerfMode.DoubleRow
WSCALE = 64.0  # host multiplies weights by this before fp8 split
AF = mybir.ActivationFunctionType
SCALE = 1.0 / 8.0  # 1/sqrt(HD)
PJ_BUFS = 1
S_BUFS = 2
O_BUFS = 2
W_BUFS = 1


def build_kernel(ctx, tc):
    nc = tc.nc
    xh = nc.dram_tensor("xh", [B, D, T], FP8, kind="ExternalInput").ap()
    xl = nc.dram_tensor("xl", [B, D, T], FP8, kind="ExternalInput").ap()
    wqh = nc.dram_tensor("wqh", [D, DQ], FP8, kind="ExternalInput").ap()
    wql = nc.dram_tensor("wql", [D, DQ], FP8, kind="ExternalInput").ap()
    wkvh = nc.dram_tensor("wkvh", [D, 2 * HD], FP8, kind="ExternalInput").ap()
    wkvl = nc.dram_tensor("wkvl", [D, 2 * HD], FP8, kind="ExternalInput").ap()
    wo = nc.dram_tensor("wo", [DQ, D], BF16, kind="ExternalInput").ap()
    out = nc.dram_tensor("out", [B, T, D], BF16, kind="ExternalOutput").ap()

    wpool = ctx.enter_context(tc.tile_pool(name="w", bufs=1))
    qt_pool = ctx.enter_context(tc.tile_pool(name="qt", bufs=2))
    kkt_pool = ctx.enter_context(tc.tile_pool(name="kkt", bufs=2))
    vt_pool = ctx.enter_context(tc.tile_pool(name="vt", bufs=2))
    v_pool = ctx.enter_context(tc.tile_pool(name="v", bufs=2))
    xt_pool = ctx.enter_context(tc.tile_pool(name="xt", bufs=6))
    p_pool = ctx.enter_context(tc.tile_pool(name="p", bufs=2))
    on_pool = ctx.enter_context(tc.tile_pool(name="on", bufs=3))
    rc_pool = ctx.enter_context(tc.tile_pool(name="rc", bufs=3))
    ot_pool = ctx.enter_context(tc.tile_pool(name="ot", bufs=2))
    stg_pool = ctx.enter_context(tc.tile_pool(name="stg", bufs=2))
    pp = ctx.enter_context(tc.tile_pool(name="pp", bufs=2, space="PSUM"))

    # persistent weights (SP/HWDGE queue; Pool is reserved for xt loads).
    # wq/wkv split into ko-chunks so the first matmuls wait only on chunk 0.
    wqh_sb = wpool.tile([P, KO, DQ], FP8, tag="wqh")
    wql_sb = wpool.tile([P, KO, DQ], FP8, tag="wql")
    wkvh_sb = wpool.tile([P, KO, 2 * HD], FP8, tag="wkvh")
    wkvl_sb = wpool.tile([P, KO, 2 * HD], FP8, tag="wkvl")
    for sb, dr in ((wqh_sb, wqh), (wkvh_sb, wkvh), (wql_sb, wql), (wkvl_sb, wkvl)):
        r = dr.rearrange("(ko p) m -> p ko m", p=P)
        for c in range(0, KO, KQ):
            nc.sync.dma_start(sb[:, c : c + KQ, :], r[:, c : c + KQ, :])
    wo_sb = wpool.tile([P, DQ // P, D], BF16, tag="wo")
    nc.sync.dma_start(wo_sb[:], wo.rearrange("(ko p) m -> p ko m", p=P))
    # upper-triangular causal mask (keep f >= p), two identical copies so one
    # tensor_tensor covers both head halves of a pair at once
    ident = wpool.tile([HD, HD], BF16, tag="ident")
    make_identity(nc, ident[:])
    tri = wpool.tile([P, 2, P], BF16, tag="tri")
    nc.gpsimd.memset(tri[:], 1.0)
    for h in range(2):
        nc.gpsimd.affine_select(
            out=tri[:, h, :],
            in_=tri[:, h, :],
            compare_op=mybir.AluOpType.is_ge,
            fill=0.0,
            base=0,
            channel_multiplier=-1,
            pattern=[[1, P]],
        )

    for b in range(B):
        qt_sb = qt_pool.tile([P, 2, T], BF16, tag="qt")  # QT, scaled by 1/8
        kkt_sb = kkt_pool.tile([P, T], BF16, tag="kkt")  # KT duplicated twice
        vt_sb = vt_pool.tile([HD, T], BF16, tag="vt")  # VT bf16 on 64 parts
        v1_sb = v_pool.tile([P, NKT, HD + 1], BF16, tag="v1")
        nc.gpsimd.memset(v1_sb[:, :, HD : HD + 1], 1.0)

        def proj(tb):
            # ---------------- projections for t-block tb ----------------
            ts = slice(tb * TB, (tb + 1) * TB)
            xhs, xls = [], []
            for src_t, lst, tag in ((xh, xhs, "xh"), (xl, xls, "xl")):
                for kq in range(KO // KQ):
                    xt = xt_pool.tile([P, KQ, TB], FP8, tag=tag)
                    nc.gpsimd.dma_start(
                        xt[:],
                        src_t[b, kq * KQ * P : (kq + 1) * KQ * P, ts].rearrange(
                            "(q p) t -> p q t", p=P
                        ),
                    )
                    lst.append(xt)
            # three sequential 1-bank accumulation chains (pair0, pair1, kv),
            # each as 3 fp8 DoubleRow passes (hi@hi + lo@hi + hi@lo) over
            # ko-pairs: error-split fp8 at 0.5 cycles/row beats bf16 on both
            # speed and accuracy. Weights are host-scaled by WSCALE so their
            # lo residuals stay in fp8 normal range; psum is WSCALE too big.
            for ci, (whsb, wlsb, lo) in enumerate(
                ((wqh_sb, wql_sb, 0), (wqh_sb, wql_sb, P), (wkvh_sb, wkvl_sb, 0))
            ):
                if b == 0 and tb == 0 and ci == 2:
                    c_ps = pp.tile([P, TB], F32, tag="O", bufs=O_BUFS)
                elif b == 0 and tb >= 1 and ci == 1:
                    c_ps = pp.tile([P, TB], F32, tag="W", bufs=W_BUFS)
                elif b == 0 and tb == 0 and ci < 2:
                    # before any attention exists the score psum is idle:
                    # borrow S slots so the first three chains overlap
                    sbig = pp.tile(
                        [P, 2, TB], F32, tag="S", bufs=S_BUFS, name=f"sb{ci}"
                    )
                    c_ps = sbig[:, 0]
                else:
                    c_ps = pp.tile([P, TB], F32, tag="PJ", bufs=PJ_BUFS)
                passes = ((whsb, xhs), (whsb, xls), (wlsb, xhs))
                n_mm = len(passes) * (KO // 2)
                i = 0
                for wsb, xlist in passes:
                    for kp in range(KO // 2):
                        ko = 2 * kp
                        nc.tensor.matmul(
                            c_ps[:],
                            wsb[:, ko : ko + 2, lo : lo + P],
                            xlist[ko // KQ][:, ko % KQ : ko % KQ + 2, :],
                            start=(i == 0),
                            stop=(i == n_mm - 1),
                            perf_mode=DR,
                        )
                        i += 1
                if ci < 2:
                    # on DVE, not ACT: keeps the exp queue free of copies
                    nc.vector.tensor_scalar_mul(
                        qt_sb[:, ci, ts], c_ps[:], SCALE / WSCALE
                    )
                else:
                    nc.vector.tensor_scalar_mul(
                        kkt_sb[0:HD, ts], c_ps[0:HD, :], 1.0 / WSCALE
                    )
                    nc.vector.tensor_scalar_mul(
                        vt_sb[:, ts], c_ps[HD:P, :], 1.0 / WSCALE
                    )
            # duplicate KT to partitions 64..127 (SBUF->SBUF DMA moves partitions)
            nc.sync.dma_start(kkt_sb[HD:P, ts], kkt_sb[0:HD, ts])
            # V transpose via PE identity matmul: [64, 128] -> [128, 64]
            # (the [64,128] xbar DMA-transpose corrupts data on HW; the
            # [128,128] o_n xbar transpose below is fine)
            for kt in range(4 * tb, 4 * tb + 4):
                tr_ps = pp.tile([P, HD], BF16, tag="W", bufs=W_BUFS)
                nc.tensor.transpose(
                    tr_ps[:], vt_sb[:, kt * P : (kt + 1) * P], ident[:]
                )
                nc.vector.tensor_copy(v1_sb[:, kt, 0:HD], tr_ps[:])

        def attn(qb):
            # ------------- attention + output proj for q-block qb -----------
            nkt = 4 * (qb + 1)  # causal: kpos tiles 0..nkt-1
            # --- phase A: scores + exp, kt-major / pair-minor: two
            # independent score->exp streams keep both S slots busy ---
            p4 = p_pool.tile([P, 2, 2, nkt, TB], BF16,
                             tag=f"P{qb % 2}", bufs=1)
            for kt in range(nkt):
                for pair in range(2):
                    p_sb = p4[:, pair]
                    ks = slice(kt * P, (kt + 1) * P)
                    dk = kt - qb * 4
                    off = max(dk, 0) * P  # first potentially-valid column
                    offc = off  # computed column start (bf16: any free size ok)
                    s_ps = pp.tile([P, 2, TB], F32, tag="S", bufs=S_BUFS)
                    qs = slice(qb * TB + offc, (qb + 1) * TB)
                    nc.tensor.matmul(
                        s_ps[:, 0, offc:],
                        kkt_sb[0:HD, ks],
                        qt_sb[0:HD, pair, qs],
                        start=True,
                        stop=True,
                        tile_position=(0, 0),
                    )
                    nc.tensor.matmul(
                        s_ps[:, 1, offc:],
                        kkt_sb[HD:P, ks],
                        qt_sb[HD:P, pair, qs],
                        start=True,
                        stop=True,
                        tile_position=(64, 0),
                    )
                    nc.scalar.activation(
                        p_sb[:, :, kt, offc:], s_ps[:, :, offc:], AF.Exp
                    )
                    if dk >= 0:  # diagonal block: causal triangle mask
                        nc.vector.tensor_mul(
                            p_sb[:, :, kt, off : off + P],
                            p_sb[:, :, kt, off : off + P],
                            tri[:],
                        )
            return p4

        def attn_b(qb, p4):
            nkt = 4 * (qb + 1)
            ot_sb = ot_pool.tile([P, 2, TB], BF16, tag="ot")
            for pair in range(2):
                p_sb = p4[:, pair]
                # --- phase B: PV accumulation, normalize, transpose ---
                for j in range(NT):
                    ktn = qb * 4 + j + 1  # kpos tiles 0..ktn-1
                    o_n = on_pool.tile([P, 2, HD], BF16, tag="on")
                    for half in range(2):
                        o_ps = pp.tile([P, HD + 1], F32, tag="O", bufs=O_BUFS)
                        for kt in range(ktn):
                            nc.tensor.matmul(
                                o_ps[:],
                                p_sb[:, half, kt, j * P : (j + 1) * P],
                                v1_sb[:, kt, :],
                                start=(kt == 0),
                                stop=(kt == ktn - 1),
                            )
                        rec = rc_pool.tile([P, 1], F32, tag="rec")
                        nc.vector.reciprocal(rec[:], o_ps[:, HD : HD + 1])
                        nc.vector.tensor_scalar_mul(
                            o_n[:, half, :], o_ps[:, 0:HD], rec[:]
                        )
                    nc.sync.dma_start_transpose(
                        ot_sb[:, pair, j * P : (j + 1) * P], o_n[:]
                    )
            return ot_sb

        def wo_block(qb, ot_sb, split_stores=False):
            # --- Wo partial for this q-block's 512 tokens ---
            for j in range(NT):
                rows = slice(qb * TB + j * P, qb * TB + (j + 1) * P)
                stg = stg_pool.tile([P, D], BF16, tag="stg")
                for nb in range(4):
                    wo_ps = pp.tile([P, TB], F32, tag="W", bufs=W_BUFS)
                    for ko in range(2):
                        nc.tensor.matmul(
                            wo_ps[:],
                            ot_sb[:, ko, j * P : (j + 1) * P],
                            wo_sb[:, ko, nb * TB : (nb + 1) * TB],
                            start=(ko == 0),
                            stop=(ko == 1),
                        )
                    nc.vector.tensor_copy(stg[:, nb * TB : (nb + 1) * TB], wo_ps[:])
                    if split_stores:
                        nc.sync.dma_start(
                            out[b, rows, nb * TB : (nb + 1) * TB],
                            stg[:, nb * TB : (nb + 1) * TB],
                        )
                if not split_stores:
                    nc.sync.dma_start(out[b, rows, :], stg[:])

        for tb in range(NT):
            proj(tb)
        # Phase A emitted one q-block ahead of phase B: the next block's
        # scores/exps outrank the previous block's PV/Wo in scheduler
        # priority, keeping the serial exp stream (the attention-phase
        # bottleneck) continuously fed.
        p1 = attn(0)
        p2 = attn(1)
        wo_block(0, attn_b(0, p1))
        p3 = attn(2)
        wo_block(1, attn_b(1, p2))
        p4_ = attn(3)
        wo_block(2, attn_b(2, p3))
        wo_block(3, attn_b(3, p4_), split_stores=(b == B - 1))


_NC_CACHE = {}


def get_nc():
    if "nc" not in _NC_CACHE:
        nc = bacc.Bacc("TRN2", target_bir_lowering=False, debug=False)
        with tile.TileContext(nc) as tc, ExitStack() as ctx:
            build_kernel(ctx, tc)
        nc.compile()
        _NC_CACHE["nc"] = nc
    return _NC_CACHE["nc"]


def make_in_maps(x, Wq, Wk, Wv, Wo):
    FP8NP = ml_dtypes.float8_e4m3

    def fp8_split(a):
        hi = a.astype(FP8NP)
        lo = (a - hi.astype(np.float32)).astype(FP8NP)
        return hi, lo

    xT = np.ascontiguousarray(np.transpose(np.asarray(x, np.float32), (0, 2, 1)))
    xh, xl = fp8_split(xT)
    Wq, Wk, Wv, Wo = (np.asarray(w, np.float32) for w in (Wq, Wk, Wv, Wo))
    in_maps = []
    for g in range(NCORES):
        in_maps.append(
            {
                "xh": xh,
                "xl": xl,
                **dict(
                    zip(
                        ("wqh", "wql"),
                        fp8_split(
                            64.0 * np.ascontiguousarray(Wq[:, g * DQ : (g + 1) * DQ])
                        ),
                    )
                ),
                **dict(
                    zip(
                        ("wkvh", "wkvl"),
                        fp8_split(
                            64.0
                            * np.ascontiguousarray(
                                np.concatenate(
                                    [
                                        Wk[:, g * HD : (g + 1) * HD],
                                        Wv[:, g * HD : (g + 1) * HD],
                                    ],
                                    axis=1,
                                )
                            )
                        ),
                    )
                ),
                "wo": np.ascontiguousarray(
                    Wo[g * DQ : (g + 1) * DQ, :]
                ).astype(ml_dtypes.bfloat16),
            }
        )
    return in_maps


def run(x, Wq, Wk, Wv, Wo, trace=False):
    nc = get_nc()
    in_maps = make_in_maps(x, Wq, Wk, Wv, Wo)
    res = run_bass_kernel_spmd(nc, in_maps, list(range(NCORES)), trace=trace)
    acc = np.zeros((B, T, D), np.float32)
    for r in res.results:
        acc += np.asarray(r["out"], dtype=np.float32)
    return acc, res


def kernel(x, Wq, Wk, Wv, Wo):
    return run(x, Wq, Wk, Wv, Wo)[0]


# revision 8
# speedup vs baseline: 1.1386x; 1.0036x over previous
"""GQA attention kernel for Trainium2, tensor-parallel across 8 NeuronCores.

Problem: B=2, T=2048, D=2048, H=32 q-heads, G=8 kv-heads (GQA, rep=4), hd=64,
causal softmax attention + output projection, fp32 I/O.

Sharding (one KV group per core):
  core g: Wq[:, g*256:(g+1)*256], Wk/Wv[:, g*64:(g+1)*64], Wo[g*256:(g+1)*256, :]
  Each core computes its 4 heads' attention + partial output projection;
  host sums the 8 partial outputs (row-parallel Wo => partial-sum unshard).
  Partial outputs are stored bf16 (halves store bandwidth); host sums in f32.

Per-core dataflow (PE cost model charges out_free_size x cycles_per_row per
matmul; bf16 = 1.0 c/r at any free size, fp8 DoubleRow = 0.5 c/r):
  Projections: fp8 error-split DoubleRow - host supplies x and (64x-scaled)
  Wq/Wkv as fp8e4 hi+lo pairs; each projection chain accumulates three
  DoubleRow passes (hi@hi + lo@hi + hi@lo) over ko-pairs, matching bf16
  accuracy at half the PE cost. Three sequential 1-bank chains (Q pair0,
  Q pair1, KV) in a dedicated psum tag; 1/64 unscaling folded into the
  psum->SBUF copies (on DVE, keeping ACT free for exps).
  K duplicated to partitions 64..127 (SBUF-SBUF DMA); V transposed to
  row-major via PE identity matmuls -> V1 [kpos, 16, hd|1] with a ones col.
  Scores per (kt, pair): two bf16 matmuls (head halves, tile_position
  quadrants) into a 2-bank psum [128, 2, 512]; ONE exp per (kt, pair) ->
  P sbuf bf16. Diagonal kt trimmed to columns >= dk*128; causal triangle
  masked by multiplying with a precomputed upper-tri bf16 mask (DVE).
  PV: per (pair, half, j): out [q=128, hd|1=65] psum accumulated over kt
  with P slices as the stationary operand (65 cycles/matmul instead of 512).
  Normalize: DVE reciprocal of col 64 ([128,1]) * cols 0..63 -> o_n bf16;
  o_n [128q, 2*64] transposed to ot [128 dq, 128 q] by DMA-engine xbar.
  Wo partial: ot.T @ wo (bf16) -> psum [128, 512] x 4 -> stg bf16
  [128, 2048] -> one DMA per 128-token row block (the last batch defers its
  final two Wo blocks into the exp-bound tail and splits the last stores).
"""

import os
import sys

import numpy as np

for _p in ("/opt/trn_rl_repo", "/root/.axon_site/_ro/trn_rl_repo"):
    if os.path.isdir(_p) and _p not in sys.path:
        sys.path.insert(0, _p)

import ml_dtypes  # noqa: E402

import concourse.bass as bass  # noqa: E402
import concourse.mybir as mybir  # noqa: E402
import concourse.tile as tile  # noqa: E402
from concourse import bacc  # noqa: E402
from concourse.bass_utils import run_bass_kernel_spmd  # noqa: E402
from concourse.masks import make_identity  # noqa: E402
from contextlib import ExitStack  # noqa: E402

B, T, D = 2, 2048, 2048
G, REP, HD = 8, 4, 64
DQ = REP * HD  # 256 q-dims per core
NCORES = 8
P = 128
TB = 512  # q/t block size
KO = D // P  # 16 contraction subtiles for projections
KQ = 4  # ko tiles per x DMA load
NT = T // TB  # 4 t-blocks
NKT = T // P  # 16 kpos tiles
F32 = mybir.dt.float32
F32R = mybir.dt.float32r
BF16 = mybir.dt.bfloat16
FP8 = mybir.dt.float8e4
DR = mybir.MatmulPerfMode.DoubleRow
WSCALE = 64.0  # host multiplies weights by this before fp8 split
AF = mybir.ActivationFunctionType
SCALE = 1.0 / 8.0  # 1/sqrt(HD)
PJ_BUFS = 1
S_BUFS = 2
O_BUFS = 2
W_BUFS = 1


def build_kernel(ctx, tc):
    nc = tc.nc
    xh = nc.dram_tensor("xh", [B, D, T], FP8, kind="ExternalInput").ap()
    xl = nc.dram_tensor("xl", [B, D, T], FP8, kind="ExternalInput").ap()
    wqh = nc.dram_tensor("wqh", [D, DQ], FP8, kind="ExternalInput").ap()
    wql = nc.dram_tensor("wql", [D, DQ], FP8, kind="ExternalInput").ap()
    wkvh = nc.dram_tensor("wkvh", [D, 2 * HD], FP8, kind="ExternalInput").ap()
    wkvl = nc.dram_tensor("wkvl", [D, 2 * HD], FP8, kind="ExternalInput").ap()
    woh = nc.dram_tensor("woh", [DQ, D], FP8, kind="ExternalInput").ap()
    wol = nc.dram_tensor("wol", [DQ, D], FP8, kind="ExternalInput").ap()
    out = nc.dram_tensor("out", [B, T, D], BF16, kind="ExternalOutput").ap()

    wpool = ctx.enter_context(tc.tile_pool(name="w", bufs=1))
    qt_pool = ctx.enter_context(tc.tile_pool(name="qt", bufs=2))
    kkt_pool = ctx.enter_context(tc.tile_pool(name="kkt", bufs=2))
    vt_pool = ctx.enter_context(tc.tile_pool(name="vt", bufs=2))
    v_pool = ctx.enter_context(tc.tile_pool(name="v", bufs=2))
    xt_pool = ctx.enter_context(tc.tile_pool(name="xt", bufs=5))
    p_pool = ctx.enter_context(tc.tile_pool(name="p", bufs=2))
    on_pool = ctx.enter_context(tc.tile_pool(name="on", bufs=3))
    rc_pool = ctx.enter_context(tc.tile_pool(name="rc", bufs=3))
    ot_pool = ctx.enter_context(tc.tile_pool(name="ot", bufs=2))
    ot8_pool = ctx.enter_context(tc.tile_pool(name="ot8", bufs=2))
    stg_pool = ctx.enter_context(tc.tile_pool(name="stg", bufs=2))
    pp = ctx.enter_context(tc.tile_pool(name="pp", bufs=2, space="PSUM"))

    # persistent weights (SP/HWDGE queue; Pool is reserved for xt loads).
    # wq/wkv split into ko-chunks so the first matmuls wait only on chunk 0.
    wqh_sb = wpool.tile([P, KO, DQ], FP8, tag="wqh")
    wql_sb = wpool.tile([P, KO, DQ], FP8, tag="wql")
    wkvh_sb = wpool.tile([P, KO, 2 * HD], FP8, tag="wkvh")
    wkvl_sb = wpool.tile([P, KO, 2 * HD], FP8, tag="wkvl")
    for sb, dr in ((wqh_sb, wqh), (wkvh_sb, wkvh), (wql_sb, wql), (wkvl_sb, wkvl)):
        r = dr.rearrange("(ko p) m -> p ko m", p=P)
        for c in range(0, KO, KQ):
            nc.sync.dma_start(sb[:, c : c + KQ, :], r[:, c : c + KQ, :])
    woh_sb = wpool.tile([P, DQ // P, D], FP8, tag="woh")
    nc.sync.dma_start(woh_sb[:], woh.rearrange("(ko p) m -> p ko m", p=P))
    wol_sb = wpool.tile([P, DQ // P, D], FP8, tag="wol")
    nc.sync.dma_start(wol_sb[:], wol.rearrange("(ko p) m -> p ko m", p=P))
    # upper-triangular causal mask (keep f >= p), two identical copies so one
    # tensor_tensor covers both head halves of a pair at once
    ident = wpool.tile([HD, HD], BF16, tag="ident")
    make_identity(nc, ident[:])
    tri = wpool.tile([P, 2, P], BF16, tag="tri")
    nc.gpsimd.memset(tri[:], 1.0)
    for h in range(2):
        nc.gpsimd.affine_select(
            out=tri[:, h, :],
            in_=tri[:, h, :],
            compare_op=mybir.AluOpType.is_ge,
            fill=0.0,
            base=0,
            channel_multiplier=-1,
            pattern=[[1, P]],
        )

    for b in range(B):
        qt_sb = qt_pool.tile([P, 2, T], BF16, tag="qt")  # QT, scaled by 1/8
        kkt_sb = kkt_pool.tile([P, T], BF16, tag="kkt")  # KT duplicated twice
        vt_sb = vt_pool.tile([HD, T], BF16, tag="vt")  # VT bf16 on 64 parts
        v1_sb = v_pool.tile([P, NKT, HD + 1], BF16, tag="v1")
        nc.gpsimd.memset(v1_sb[:, :, HD : HD + 1], 1.0)

        def proj(tb):
            # ---------------- projections for t-block tb ----------------
            ts = slice(tb * TB, (tb + 1) * TB)
            xhs, xls = [], []
            for src_t, lst, tag in ((xh, xhs, "xh"), (xl, xls, "xl")):
                for kq in range(KO // KQ):
                    xt = xt_pool.tile([P, KQ, TB], FP8, tag=tag)
                    nc.gpsimd.dma_start(
                        xt[:],
                        src_t[b, kq * KQ * P : (kq + 1) * KQ * P, ts].rearrange(
                            "(q p) t -> p q t", p=P
                        ),
                    )
                    lst.append(xt)
            # three sequential 1-bank accumulation chains (pair0, pair1, kv),
            # each as 3 fp8 DoubleRow passes (hi@hi + lo@hi + hi@lo) over
            # ko-pairs: error-split fp8 at 0.5 cycles/row beats bf16 on both
            # speed and accuracy. Weights are host-scaled by WSCALE so their
            # lo residuals stay in fp8 normal range; psum is WSCALE too big.
            for ci, (whsb, wlsb, lo) in enumerate(
                ((wqh_sb, wql_sb, 0), (wqh_sb, wql_sb, P), (wkvh_sb, wkvl_sb, 0))
            ):
                if b == 0 and tb == 0 and ci == 2:
                    c_ps = pp.tile([P, TB], F32, tag="O", bufs=O_BUFS)
                elif b == 0 and tb >= 1 and ci == 1:
                    c_ps = pp.tile([P, TB], F32, tag="W", bufs=W_BUFS)
                elif b == 0 and tb == 0 and ci < 2:
                    # before any attention exists the score psum is idle:
                    # borrow S slots so the first three chains overlap
                    sbig = pp.tile(
                        [P, 2, TB], F32, tag="S", bufs=S_BUFS, name=f"sb{ci}"
                    )
                    c_ps = sbig[:, 0]
                else:
                    c_ps = pp.tile([P, TB], F32, tag="PJ", bufs=PJ_BUFS)
                passes = ((whsb, xhs), (whsb, xls), (wlsb, xhs))
                n_mm = len(passes) * (KO // 2)
                i = 0
                for wsb, xlist in passes:
                    for kp in range(KO // 2):
                        ko = 2 * kp
                        nc.tensor.matmul(
                            c_ps[:],
                            wsb[:, ko : ko + 2, lo : lo + P],
                            xlist[ko // KQ][:, ko % KQ : ko % KQ + 2, :],
                            start=(i == 0),
                            stop=(i == n_mm - 1),
                            perf_mode=DR,
                        )
                        i += 1
                if ci < 2:
                    # on DVE, not ACT: keeps the exp queue free of copies
                    nc.vector.tensor_scalar_mul(
                        qt_sb[:, ci, ts], c_ps[:], SCALE / WSCALE
                    )
                else:
                    nc.vector.tensor_scalar_mul(
                        kkt_sb[0:HD, ts], c_ps[0:HD, :], 1.0 / WSCALE
                    )
                    nc.vector.tensor_scalar_mul(
                        vt_sb[:, ts], c_ps[HD:P, :], 1.0 / WSCALE
                    )
            # duplicate KT to partitions 64..127 (SBUF->SBUF DMA moves partitions)
            nc.sync.dma_start(kkt_sb[HD:P, ts], kkt_sb[0:HD, ts])
            # V transpose via PE identity matmul: [64, 128] -> [128, 64]
            # (the [64,128] xbar DMA-transpose corrupts data on HW; the
            # [128,128] o_n xbar transpose below is fine)
            for kt in range(4 * tb, 4 * tb + 4):
                tr_ps = pp.tile([P, HD], BF16, tag="W", bufs=W_BUFS)
                nc.tensor.transpose(
                    tr_ps[:], vt_sb[:, kt * P : (kt + 1) * P], ident[:]
                )
                nc.vector.tensor_copy(v1_sb[:, kt, 0:HD], tr_ps[:])

        def attn(qb):
            # ------------- attention + output proj for q-block qb -----------
            nkt = 4 * (qb + 1)  # causal: kpos tiles 0..nkt-1
            # --- phase A: scores + exp, kt-major / pair-minor: two
            # independent score->exp streams keep both S slots busy ---
            p4 = p_pool.tile([P, 2, 2, nkt, TB], BF16,
                             tag=f"P{qb % 2}", bufs=1)
            for kt in range(nkt):
                for pair in range(2):
                    p_sb = p4[:, pair]
                    ks = slice(kt * P, (kt + 1) * P)
                    dk = kt - qb * 4
                    off = max(dk, 0) * P  # first potentially-valid column
                    offc = off  # computed column start (bf16: any free size ok)
                    s_ps = pp.tile([P, 2, TB], F32, tag="S", bufs=S_BUFS)
                    qs = slice(qb * TB + offc, (qb + 1) * TB)
                    nc.tensor.matmul(
                        s_ps[:, 0, offc:],
                        kkt_sb[0:HD, ks],
                        qt_sb[0:HD, pair, qs],
                        start=True,
                        stop=True,
                        tile_position=(0, 0),
                    )
                    nc.tensor.matmul(
                        s_ps[:, 1, offc:],
                        kkt_sb[HD:P, ks],
                        qt_sb[HD:P, pair, qs],
                        start=True,
                        stop=True,
                        tile_position=(64, 0),
                    )
                    nc.scalar.activation(
                        p_sb[:, :, kt, offc:], s_ps[:, :, offc:], AF.Exp
                    )
                    if dk >= 0:  # diagonal block: causal triangle mask
                        nc.vector.tensor_mul(
                            p_sb[:, :, kt, off : off + P],
                            p_sb[:, :, kt, off : off + P],
                            tri[:],
                        )
            return p4

        def attn_b(qb, p4):
            nkt = 4 * (qb + 1)
            ot_sb = ot_pool.tile([P, 2, TB], BF16, tag="ot")
            ot8h = ot8_pool.tile([P, 2, TB], FP8, tag="oh")
            ot8l = ot8_pool.tile([P, 2, TB], FP8, tag="ol")
            for pair in range(2):
                p_sb = p4[:, pair]
                # --- phase B: PV accumulation, normalize, transpose ---
                for j in range(NT):
                    ktn = qb * 4 + j + 1  # kpos tiles 0..ktn-1
                    o_n = on_pool.tile([P, 2, HD], BF16, tag="on")
                    for half in range(2):
                        o_ps = pp.tile([P, HD + 1], F32, tag="O", bufs=O_BUFS)
                        for kt in range(ktn):
                            nc.tensor.matmul(
                                o_ps[:],
                                p_sb[:, half, kt, j * P : (j + 1) * P],
                                v1_sb[:, kt, :],
                                start=(kt == 0),
                                stop=(kt == ktn - 1),
                            )
                        rec = rc_pool.tile([P, 1], F32, tag="rec")
                        nc.vector.reciprocal(rec[:], o_ps[:, HD : HD + 1])
                        nc.vector.tensor_scalar(
                            o_n[:, half, :], o_ps[:, 0:HD], rec[:], 8.0,
                            mybir.AluOpType.mult, mybir.AluOpType.mult,
                        )
                    js = slice(j * P, (j + 1) * P)
                    nc.sync.dma_start_transpose(ot_sb[:, pair, js], o_n[:])
                    nc.vector.tensor_copy(
                        ot8h[:, pair, js], ot_sb[:, pair, js]
                    )
                    nc.vector.tensor_sub(
                        ot8l[:, pair, js], ot_sb[:, pair, js], ot8h[:, pair, js]
                    )
            return ot8h, ot8l

        def wo_block(qb, ots, split_stores=False):
            ot8h, ot8l = ots
            # --- Wo partial for this q-block's 512 tokens ---
            for j in range(NT):
                rows = slice(qb * TB + j * P, qb * TB + (j + 1) * P)
                stg = stg_pool.tile([P, D], BF16, tag="stg")
                for nb in range(4):
                    wo_ps = pp.tile([P, TB], F32, tag="W", bufs=W_BUFS)
                    ns = slice(nb * TB, (nb + 1) * TB)
                    for i, (osb, wsb) in enumerate(
                        ((ot8h, woh_sb), (ot8l, woh_sb), (ot8h, wol_sb))
                    ):
                        nc.tensor.matmul(
                            wo_ps[:],
                            osb[:, :, j * P : (j + 1) * P],
                            wsb[:, :, ns],
                            start=(i == 0),
                            stop=(i == 2),
                            perf_mode=DR,
                        )
                    nc.vector.tensor_scalar_mul(
                        stg[:, ns], wo_ps[:], 1.0 / (8.0 * WSCALE)
                    )
                    if split_stores:
                        nc.sync.dma_start(
                            out[b, rows, nb * TB : (nb + 1) * TB],
                            stg[:, nb * TB : (nb + 1) * TB],
                        )
                if not split_stores:
                    nc.sync.dma_start(out[b, rows, :], stg[:])

        for tb in range(NT):
            proj(tb)
        # Phase A emitted one q-block ahead of phase B: the next block's
        # scores/exps outrank the previous block's PV/Wo in scheduler
        # priority, keeping the serial exp stream (the attention-phase
        # bottleneck) continuously fed.
        p1 = attn(0)
        p2 = attn(1)
        wo_block(0, attn_b(0, p1))
        p3 = attn(2)
        wo_block(1, attn_b(1, p2))
        p4_ = attn(3)
        wo_block(2, attn_b(2, p3))
        wo_block(3, attn_b(3, p4_), split_stores=(b == B - 1))


_NC_CACHE = {}


def get_nc():
    if "nc" not in _NC_CACHE:
        nc = bacc.Bacc("TRN2", target_bir_lowering=False, debug=False)
        with tile.TileContext(nc) as tc, ExitStack() as ctx:
            build_kernel(ctx, tc)
        nc.compile()
        _NC_CACHE["nc"] = nc
    return _NC_CACHE["nc"]


def make_in_maps(x, Wq, Wk, Wv, Wo):
    FP8NP = ml_dtypes.float8_e4m3

    def fp8_split(a):
        hi = a.astype(FP8NP)
        lo = (a - hi.astype(np.float32)).astype(FP8NP)
        return hi, lo

    xT = np.ascontiguousarray(np.transpose(np.asarray(x, np.float32), (0, 2, 1)))
    xh, xl = fp8_split(xT)
    Wq, Wk, Wv, Wo = (np.asarray(w, np.float32) for w in (Wq, Wk, Wv, Wo))
    in_maps = []
    for g in range(NCORES):
        in_maps.append(
            {
                "xh": xh,
                "xl": xl,
                **dict(
                    zip(
                        ("wqh", "wql"),
                        fp8_split(
                            64.0 * np.ascontiguousarray(Wq[:, g * DQ : (g + 1) * DQ])
                        ),
                    )
                ),
                **dict(
                    zip(
                        ("wkvh", "wkvl"),
                        fp8_split(
                            64.0
                            * np.ascontiguousarray(
                                np.concatenate(
                                    [
                                        Wk[:, g * HD : (g + 1) * HD],
                                        Wv[:, g * HD : (g + 1) * HD],
                                    ],
                                    axis=1,
                                )
                            )
                        ),
                    )
                ),
                **dict(
                    zip(
                        ("woh", "wol"),
                        fp8_split(
                            64.0
                            * np.ascontiguousarray(Wo[g * DQ : (g + 1) * DQ, :])
                        ),
                    )
                ),
            }
        )
    return in_maps


def run(x, Wq, Wk, Wv, Wo, trace=False):
    nc = get_nc()
    in_maps = make_in_maps(x, Wq, Wk, Wv, Wo)
    res = run_bass_kernel_spmd(nc, in_maps, list(range(NCORES)), trace=trace)
    acc = np.zeros((B, T, D), np.float32)
    for r in res.results:
        acc += np.asarray(r["out"], dtype=np.float32)
    return acc, res


def kernel(x, Wq, Wk, Wv, Wo):
    return run(x, Wq, Wk, Wv, Wo)[0]


# revision 9
# speedup vs baseline: 1.1622x; 1.0207x over previous
"""GQA attention kernel for Trainium2, tensor-parallel across 8 NeuronCores.

Problem: B=2, T=2048, D=2048, H=32 q-heads, G=8 kv-heads (GQA, rep=4), hd=64,
causal softmax attention + output projection, fp32 I/O.

Sharding (one KV group per core):
  core g: Wq[:, g*256:(g+1)*256], Wk/Wv[:, g*64:(g+1)*64], Wo[g*256:(g+1)*256, :]
  Each core computes its 4 heads' attention + partial output projection;
  host sums the 8 partial outputs (row-parallel Wo => partial-sum unshard).
  Partial outputs are stored bf16 (halves store bandwidth); host sums in f32.

Per-core dataflow (PE cost model charges out_free_size x cycles_per_row per
matmul; bf16 = 1.0 c/r at any free size, fp8 DoubleRow = 0.5 c/r):
  Projections: fp8 error-split DoubleRow - host supplies x and (64x-scaled)
  Wq/Wkv as fp8e4 hi+lo pairs; each projection chain accumulates three
  DoubleRow passes (hi@hi + lo@hi + hi@lo) over ko-pairs, matching bf16
  accuracy at half the PE cost. Three sequential 1-bank chains (Q pair0,
  Q pair1, KV) in a dedicated psum tag; 1/64 unscaling folded into the
  psum->SBUF copies (on DVE, keeping ACT free for exps).
  K duplicated to partitions 64..127 (SBUF-SBUF DMA); V transposed to
  row-major via PE identity matmuls -> V1 [kpos, 16, hd|1] with a ones col.
  Scores per (kt, pair): two bf16 matmuls (head halves, tile_position
  quadrants) into a 2-bank psum [128, 2, 512]; ONE exp per (kt, pair) ->
  P sbuf bf16. Diagonal kt trimmed to columns >= dk*128; causal triangle
  masked by multiplying with a precomputed upper-tri bf16 mask (DVE).
  PV: per (pair, half, j): out [q=128, hd|1=65] psum accumulated over kt
  with P slices as the stationary operand (65 cycles/matmul instead of 512).
  Normalize: DVE reciprocal of col 64 ([128,1]) * cols 0..63 -> o_n bf16;
  o_n [128q, 2*64] transposed to ot [128 dq, 128 q] by DMA-engine xbar.
  Wo partial: ot.T @ wo (bf16) -> psum [128, 512] x 4 -> stg bf16
  [128, 2048] -> one DMA per 128-token row block (the last batch defers its
  final two Wo blocks into the exp-bound tail and splits the last stores).
"""

import os
import sys

import numpy as np

for _p in ("/opt/trn_rl_repo", "/root/.axon_site/_ro/trn_rl_repo"):
    if os.path.isdir(_p) and _p not in sys.path:
        sys.path.insert(0, _p)

import ml_dtypes  # noqa: E402

import concourse.bass as bass  # noqa: E402
import concourse.mybir as mybir  # noqa: E402
import concourse.tile as tile  # noqa: E402
from concourse import bacc  # noqa: E402
from concourse.bass_utils import run_bass_kernel_spmd  # noqa: E402
from concourse.masks import make_identity  # noqa: E402
from contextlib import ExitStack  # noqa: E402

B, T, D = 2, 2048, 2048
G, REP, HD = 8, 4, 64
DQ = REP * HD  # 256 q-dims per core
NCORES = 8
P = 128
TB = 512  # q/t block size
KO = D // P  # 16 contraction subtiles for projections
KQ = 4  # ko tiles per x DMA load
NT = T // TB  # 4 t-blocks
NKT = T // P  # 16 kpos tiles
F32 = mybir.dt.float32
F32R = mybir.dt.float32r
BF16 = mybir.dt.bfloat16
FP8 = mybir.dt.float8e4
DR = mybir.MatmulPerfMode.DoubleRow
WSCALE = 64.0  # host multiplies weights by this before fp8 split
AF = mybir.ActivationFunctionType
SCALE = 1.0 / 8.0  # 1/sqrt(HD)
PJ_BUFS = 1
S_BUFS = 2
O_BUFS = 2
W_BUFS = 1


def build_kernel(ctx, tc):
    nc = tc.nc
    xh = nc.dram_tensor("xh", [B, D, T], FP8, kind="ExternalInput").ap()
    xl = nc.dram_tensor("xl", [B, D, T], FP8, kind="ExternalInput").ap()
    wqh = nc.dram_tensor("wqh", [D, DQ], FP8, kind="ExternalInput").ap()
    wql = nc.dram_tensor("wql", [D, DQ], FP8, kind="ExternalInput").ap()
    wkvh = nc.dram_tensor("wkvh", [D, 2 * HD], FP8, kind="ExternalInput").ap()
    wkvl = nc.dram_tensor("wkvl", [D, 2 * HD], FP8, kind="ExternalInput").ap()
    woh = nc.dram_tensor("woh", [DQ, D], FP8, kind="ExternalInput").ap()
    wol = nc.dram_tensor("wol", [DQ, D], FP8, kind="ExternalInput").ap()
    out = nc.dram_tensor("out", [B, T, D], BF16, kind="ExternalOutput").ap()

    wpool = ctx.enter_context(tc.tile_pool(name="w", bufs=1))
    qt_pool = ctx.enter_context(tc.tile_pool(name="qt", bufs=2))
    kkt_pool = ctx.enter_context(tc.tile_pool(name="kkt", bufs=2))
    vt_pool = ctx.enter_context(tc.tile_pool(name="vt", bufs=2))
    v_pool = ctx.enter_context(tc.tile_pool(name="v", bufs=2))
    xt_pool = ctx.enter_context(tc.tile_pool(name="xt", bufs=5))
    p_pool = ctx.enter_context(tc.tile_pool(name="p", bufs=2))
    on_pool = ctx.enter_context(tc.tile_pool(name="on", bufs=3))
    rc_pool = ctx.enter_context(tc.tile_pool(name="rc", bufs=3))
    ot_pool = ctx.enter_context(tc.tile_pool(name="ot", bufs=2))
    ot8_pool = ctx.enter_context(tc.tile_pool(name="ot8", bufs=2))
    stg_pool = ctx.enter_context(tc.tile_pool(name="stg", bufs=2))
    pp = ctx.enter_context(tc.tile_pool(name="pp", bufs=2, space="PSUM"))

    # persistent weights (SP/HWDGE queue; Pool is reserved for xt loads).
    # wq/wkv split into ko-chunks so the first matmuls wait only on chunk 0.
    wqh_sb = wpool.tile([P, KO, DQ], FP8, tag="wqh")
    wql_sb = wpool.tile([P, KO, DQ], FP8, tag="wql")
    wkvh_sb = wpool.tile([P, KO, 2 * HD], FP8, tag="wkvh")
    wkvl_sb = wpool.tile([P, KO, 2 * HD], FP8, tag="wkvl")
    for sb, dr in ((wqh_sb, wqh), (wkvh_sb, wkvh), (wql_sb, wql), (wkvl_sb, wkvl)):
        r = dr.rearrange("(ko p) m -> p ko m", p=P)
        for c in range(0, KO, KQ):
            nc.sync.dma_start(sb[:, c : c + KQ, :], r[:, c : c + KQ, :])
    woh_sb = wpool.tile([P, DQ // P, D], FP8, tag="woh")
    nc.sync.dma_start(woh_sb[:], woh.rearrange("(ko p) m -> p ko m", p=P))
    wol_sb = wpool.tile([P, DQ // P, D], FP8, tag="wol")
    nc.sync.dma_start(wol_sb[:], wol.rearrange("(ko p) m -> p ko m", p=P))
    # upper-triangular causal mask (keep f >= p), two identical copies so one
    # tensor_tensor covers both head halves of a pair at once
    ident = wpool.tile([HD, HD], BF16, tag="ident")
    make_identity(nc, ident[:])
    tri = wpool.tile([P, 2, P], BF16, tag="tri")
    nc.gpsimd.memset(tri[:], 1.0)
    for h in range(2):
        nc.gpsimd.affine_select(
            out=tri[:, h, :],
            in_=tri[:, h, :],
            compare_op=mybir.AluOpType.is_ge,
            fill=0.0,
            base=0,
            channel_multiplier=-1,
            pattern=[[1, P]],
        )

    for b in range(B):
        qt_sb = qt_pool.tile([P, 2, T], BF16, tag="qt")  # QT, scaled by 1/8
        kkt_sb = kkt_pool.tile([P, T], BF16, tag="kkt")  # KT duplicated twice
        vt_sb = vt_pool.tile([HD, T], BF16, tag="vt")  # VT bf16 on 64 parts
        v1_sb = v_pool.tile([P, NKT, HD + 1], BF16, tag="v1")
        nc.gpsimd.memset(v1_sb[:, :, HD : HD + 1], 1.0)

        def proj(tb):
            # ---------------- projections for t-block tb ----------------
            ts = slice(tb * TB, (tb + 1) * TB)
            xhs, xls = [], []
            for src_t, lst, tag in ((xh, xhs, "xh"), (xl, xls, "xl")):
                for kq in range(KO // KQ):
                    xt = xt_pool.tile([P, KQ, TB], FP8, tag=tag)
                    nc.gpsimd.dma_start(
                        xt[:],
                        src_t[b, kq * KQ * P : (kq + 1) * KQ * P, ts].rearrange(
                            "(q p) t -> p q t", p=P
                        ),
                    )
                    lst.append(xt)
            # three sequential 1-bank accumulation chains (pair0, pair1, kv),
            # each as 3 fp8 DoubleRow passes (hi@hi + lo@hi + hi@lo) over
            # ko-pairs: error-split fp8 at 0.5 cycles/row beats bf16 on both
            # speed and accuracy. Weights are host-scaled by WSCALE so their
            # lo residuals stay in fp8 normal range; psum is WSCALE too big.
            for ci, (whsb, wlsb, lo) in enumerate(
                ((wqh_sb, wql_sb, 0), (wqh_sb, wql_sb, P), (wkvh_sb, wkvl_sb, 0))
            ):
                if b == 0 and tb == 0 and ci == 2:
                    c_ps = pp.tile([P, TB], F32, tag="O", bufs=O_BUFS)
                elif b == 0 and tb >= 1 and ci == 1:
                    c_ps = pp.tile([P, TB], F32, tag="W", bufs=W_BUFS)
                elif b == 0 and tb == 0 and ci < 2:
                    # before any attention exists the score psum is idle:
                    # borrow S slots so the first three chains overlap
                    sbig = pp.tile(
                        [P, 2, TB], F32, tag="S", bufs=S_BUFS, name=f"sb{ci}"
                    )
                    c_ps = sbig[:, 0]
                else:
                    c_ps = pp.tile([P, TB], F32, tag="PJ", bufs=PJ_BUFS)
                passes = ((whsb, xhs), (whsb, xls), (wlsb, xhs))
                n_mm = len(passes) * (KO // 2)
                i = 0
                for wsb, xlist in passes:
                    for kp in range(KO // 2):
                        ko = 2 * kp
                        nc.tensor.matmul(
                            c_ps[:],
                            wsb[:, ko : ko + 2, lo : lo + P],
                            xlist[ko // KQ][:, ko % KQ : ko % KQ + 2, :],
                            start=(i == 0),
                            stop=(i == n_mm - 1),
                            perf_mode=DR,
                        )
                        i += 1
                if ci < 2:
                    # on DVE, not ACT: keeps the exp queue free of copies
                    nc.vector.tensor_scalar_mul(
                        qt_sb[:, ci, ts], c_ps[:], SCALE / WSCALE
                    )
                else:
                    nc.vector.tensor_scalar_mul(
                        kkt_sb[0:HD, ts], c_ps[0:HD, :], 1.0 / WSCALE
                    )
                    nc.vector.tensor_scalar_mul(
                        vt_sb[:, ts], c_ps[HD:P, :], 1.0 / WSCALE
                    )
            # duplicate KT to partitions 64..127 (SBUF->SBUF DMA moves partitions)
            nc.sync.dma_start(kkt_sb[HD:P, ts], kkt_sb[0:HD, ts])
            # V transpose via PE identity matmul: [64, 128] -> [128, 64]
            # (the [64,128] xbar DMA-transpose corrupts data on HW; the
            # [128,128] o_n xbar transpose below is fine)
            for kt in range(4 * tb, 4 * tb + 4):
                tr_ps = pp.tile([P, HD], BF16, tag="W", bufs=W_BUFS)
                nc.tensor.transpose(
                    tr_ps[:], vt_sb[:, kt * P : (kt + 1) * P], ident[:]
                )
                nc.vector.tensor_copy(v1_sb[:, kt, 0:HD], tr_ps[:])

        def attn(qb):
            # ------------- attention + output proj for q-block qb -----------
            nkt = 4 * (qb + 1)  # causal: kpos tiles 0..nkt-1
            # --- phase A: scores + exp, kt-major / pair-minor: two
            # independent score->exp streams keep both S slots busy ---
            p4 = p_pool.tile([P, 2, 2, nkt, TB], BF16,
                             tag=f"P{qb % 2}", bufs=1)
            for kt in range(nkt):
                for pair in range(2):
                    p_sb = p4[:, pair]
                    ks = slice(kt * P, (kt + 1) * P)
                    dk = kt - qb * 4
                    off = max(dk, 0) * P  # first potentially-valid column
                    offc = off  # computed column start (bf16: any free size ok)
                    s_ps = pp.tile([P, 2, TB], F32, tag="S", bufs=S_BUFS)
                    qs = slice(qb * TB + offc, (qb + 1) * TB)
                    nc.tensor.matmul(
                        s_ps[:, 0, offc:],
                        kkt_sb[0:HD, ks],
                        qt_sb[0:HD, pair, qs],
                        start=True,
                        stop=True,
                        tile_position=(0, 0),
                    )
                    nc.tensor.matmul(
                        s_ps[:, 1, offc:],
                        kkt_sb[HD:P, ks],
                        qt_sb[HD:P, pair, qs],
                        start=True,
                        stop=True,
                        tile_position=(64, 0),
                    )
                    nc.scalar.activation(
                        p_sb[:, :, kt, offc:], s_ps[:, :, offc:], AF.Exp
                    )
                    if dk >= 0:  # diagonal block: causal triangle mask
                        nc.vector.tensor_mul(
                            p_sb[:, :, kt, off : off + P],
                            p_sb[:, :, kt, off : off + P],
                            tri[:],
                        )
            return p4

        def attn_b(qb, p4):
            nkt = 4 * (qb + 1)
            ot_sb = ot_pool.tile([P, 2, TB], BF16, tag="ot")
            ot8h = ot8_pool.tile([P, 2, TB], FP8, tag="oh")
            ot8l = ot8_pool.tile([P, 2, TB], FP8, tag="ol")
            for pair in range(2):
                p_sb = p4[:, pair]
                # --- phase B: PV accumulation, normalize, transpose ---
                for j in range(NT):
                    ktn = qb * 4 + j + 1  # kpos tiles 0..ktn-1
                    o_n = on_pool.tile([P, 2, HD], BF16, tag="on")
                    for half in range(2):
                        o_ps = pp.tile([P, HD + 1], F32, tag="O", bufs=O_BUFS)
                        for kt in range(ktn):
                            nc.tensor.matmul(
                                o_ps[:],
                                p_sb[:, half, kt, j * P : (j + 1) * P],
                                v1_sb[:, kt, :],
                                start=(kt == 0),
                                stop=(kt == ktn - 1),
                            )
                        rec = rc_pool.tile([P, 1], F32, tag="rec")
                        nc.vector.reciprocal(rec[:], o_ps[:, HD : HD + 1])
                        nc.vector.tensor_scalar(
                            o_n[:, half, :], o_ps[:, 0:HD], rec[:], 8.0,
                            mybir.AluOpType.mult, mybir.AluOpType.mult,
                        )
                    js = slice(j * P, (j + 1) * P)
                    nc.sync.dma_start_transpose(ot_sb[:, pair, js], o_n[:])
                    nc.vector.tensor_copy(
                        ot8h[:, pair, js], ot_sb[:, pair, js]
                    )
                    nc.vector.tensor_sub(
                        ot8l[:, pair, js], ot_sb[:, pair, js], ot8h[:, pair, js]
                    )
            return ot8h, ot8l

        def wo_block(qb, ots, split_stores=False):
            ot8h, ot8l = ots
            # --- Wo partial for this q-block's 512 tokens ---
            for j in range(NT):
                rows = slice(qb * TB + j * P, qb * TB + (j + 1) * P)
                stg = stg_pool.tile([P, D], BF16, tag="stg")
                for nb in range(4):
                    wo_ps = pp.tile([P, TB], F32, tag="W", bufs=W_BUFS)
                    ns = slice(nb * TB, (nb + 1) * TB)
                    for i, (osb, wsb) in enumerate(
                        ((ot8h, woh_sb), (ot8l, woh_sb), (ot8h, wol_sb))
                    ):
                        nc.tensor.matmul(
                            wo_ps[:],
                            osb[:, :, j * P : (j + 1) * P],
                            wsb[:, :, ns],
                            start=(i == 0),
                            stop=(i == 2),
                            perf_mode=DR,
                        )
                    if split_stores:
                        # final block: ACT is idle (no exps left); keep the
                        # drain chain off the busy DVE queue
                        nc.scalar.mul(stg[:, ns], wo_ps[:], 1.0 / (8.0 * WSCALE))
                    else:
                        nc.vector.tensor_scalar_mul(
                            stg[:, ns], wo_ps[:], 1.0 / (8.0 * WSCALE)
                        )
                    if split_stores:
                        nc.sync.dma_start(
                            out[b, rows, nb * TB : (nb + 1) * TB],
                            stg[:, nb * TB : (nb + 1) * TB],
                        )
                if not split_stores:
                    nc.sync.dma_start(out[b, rows, :], stg[:])

        for tb in range(NT):
            proj(tb)
        # Phase A emitted one q-block ahead of phase B: the next block's
        # scores/exps outrank the previous block's PV/Wo in scheduler
        # priority, keeping the serial exp stream (the attention-phase
        # bottleneck) continuously fed.
        p1 = attn(0)
        p2 = attn(1)
        wo_block(0, attn_b(0, p1))
        p3 = attn(2)
        wo_block(1, attn_b(1, p2))
        p4_ = attn(3)
        wo_block(2, attn_b(2, p3))
        wo_block(3, attn_b(3, p4_), split_stores=(b == B - 1))


_NC_CACHE = {}


def get_nc():
    if "nc" not in _NC_CACHE:
        nc = bacc.Bacc("TRN2", target_bir_lowering=False, debug=False)
        with tile.TileContext(nc) as tc, ExitStack() as ctx:
            build_kernel(ctx, tc)
        nc.compile()
        _NC_CACHE["nc"] = nc
    return _NC_CACHE["nc"]


def make_in_maps(x, Wq, Wk, Wv, Wo):
    FP8NP = ml_dtypes.float8_e4m3

    def fp8_split(a):
        hi = a.astype(FP8NP)
        lo = (a - hi.astype(np.float32)).astype(FP8NP)
        return hi, lo

    xT = np.ascontiguousarray(np.transpose(np.asarray(x, np.float32), (0, 2, 1)))
    xh, xl = fp8_split(xT)
    Wq, Wk, Wv, Wo = (np.asarray(w, np.float32) for w in (Wq, Wk, Wv, Wo))
    in_maps = []
    for g in range(NCORES):
        in_maps.append(
            {
                "xh": xh,
                "xl": xl,
                **dict(
                    zip(
                        ("wqh", "wql"),
                        fp8_split(
                            64.0 * np.ascontiguousarray(Wq[:, g * DQ : (g + 1) * DQ])
                        ),
                    )
                ),
                **dict(
                    zip(
                        ("wkvh", "wkvl"),
                        fp8_split(
                            64.0
                            * np.ascontiguousarray(
                                np.concatenate(
                                    [
                                        Wk[:, g * HD : (g + 1) * HD],
                                        Wv[:, g * HD : (g + 1) * HD],
                                    ],
                                    axis=1,
                                )
                            )
                        ),
                    )
                ),
                **dict(
                    zip(
                        ("woh", "wol"),
                        fp8_split(
                            64.0
                            * np.ascontiguousarray(Wo[g * DQ : (g + 1) * DQ, :])
                        ),
                    )
                ),
            }
        )
    return in_maps


def run(x, Wq, Wk, Wv, Wo, trace=False):
    nc = get_nc()
    in_maps = make_in_maps(x, Wq, Wk, Wv, Wo)
    res = run_bass_kernel_spmd(nc, in_maps, list(range(NCORES)), trace=trace)
    acc = np.zeros((B, T, D), np.float32)
    for r in res.results:
        acc += np.asarray(r["out"], dtype=np.float32)
    return acc, res


def kernel(x, Wq, Wk, Wv, Wo):
    return run(x, Wq, Wk, Wv, Wo)[0]
